# revision 3
# baseline (speedup 1.0000x reference)
"""Trainium2 Bass kernel for nn_AttentionBlock (B=4, H=W=64, C=64, GroupNorm(8) +
full spatial self-attention), distributed over 8 NeuronCores.

Sharding: core i handles batch b=i//2 and query-half h=i%2 (2048 of the 4096
spatial positions). Each core computes the full GroupNorm and K/V for its
image (cheap) and attention only for its query half. No collectives.

Device layout: channel-on-partition ("xT") layout, with the two position
halves of an image packed onto partitions [(half, channel)] -> 128 partitions.
Scores are computed transposed (positions' on partitions) so that
- softmax reduction over positions' is a PE matmul (ones column in V), and
- the attention @ V contraction needs no transposes at all.
exp() runs on ScalarE reading PSUM and writing SBUF directly.
"""

import sys

sys.path.insert(0, "/opt/trn_rl_repo")

import numpy as np

import concourse.bacc as bacc
import concourse.tile as tile
from concourse import mybir

B, H, W, C = 4, 64, 64, 64
HW = H * W  # 4096
HALF = HW // 2  # 2048
EPS = 1e-5
SCALE = C ** -0.5  # folded into exp()

F32 = mybir.dt.float32
# dtype used for the PE matmuls. float32r runs the PE at 4x the float32 rate
# (1 cycle/row at N>=256) with ~tf32 precision.
MM_DT = mybir.dt.float32

EXP_BATCH = 3  # pos'-chunks (PSUM banks) per exp() call


def _mm(ap):
    """View an fp32 AP as MM_DT for TensorE (same 32-bit storage)."""
    if MM_DT == F32:
        return ap
    return ap.bitcast(MM_DT)


def build_nc():
    nc = bacc.Bacc("TRN2", debug=False, num_devices=8)

    # ---- DRAM I/O ----
    xp_d = nc.dram_tensor("xp", [128, HALF], F32, kind="ExternalInput")
    wq_d = nc.dram_tensor("wq", [64, 128], F32, kind="ExternalInput")
    wk_d = nc.dram_tensor("wk", [128, 128], F32, kind="ExternalInput")
    wv_d = nc.dram_tensor("wv", [128, 128], F32, kind="ExternalInput")
    wo_d = nc.dram_tensor("wo", [64, 64], F32, kind="ExternalInput")
    bq_d = nc.dram_tensor("bq", [1, 128], F32, kind="ExternalInput")
    bk_d = nc.dram_tensor("bk", [1, 128], F32, kind="ExternalInput")
    bv_d = nc.dram_tensor("bv", [1, 128], F32, kind="ExternalInput")
    bo_d = nc.dram_tensor("bo", [1, 64], F32, kind="ExternalInput")
    gam_d = nc.dram_tensor("gam", [128, 1], F32, kind="ExternalInput")
    bet_d = nc.dram_tensor("bet", [128, 1], F32, kind="ExternalInput")
    comb_d = nc.dram_tensor("comb", [128, 128], F32, kind="ExternalInput")
    out_d = nc.dram_tensor("out", [64, HALF], F32, kind="ExternalOutput")

    with tile.TileContext(nc) as tc, \
         tc.tile_pool(name="singles", bufs=1) as singles, \
         tc.tile_pool(name="stats", bufs=4) as stats:

        # ---- constant loads ----
        x_sb = singles.tile([128, HALF], F32)
        nc.sync.dma_start(x_sb[:], xp_d.ap())
        wq_sb = singles.tile([64, 128], F32)
        nc.sync.dma_start(wq_sb[:], wq_d.ap())
        wk_sb = singles.tile([128, 128], F32)
        nc.sync.dma_start(wk_sb[:], wk_d.ap())
        wv_sb = singles.tile([128, 128], F32)
        nc.sync.dma_start(wv_sb[:], wv_d.ap())
        wo_sb = singles.tile([64, 64], F32)
        nc.sync.dma_start(wo_sb[:], wo_d.ap())
        bq_sb = singles.tile([1, 128], F32)
        nc.sync.dma_start(bq_sb[:], bq_d.ap())
        bk_sb = singles.tile([1, 128], F32)
        nc.sync.dma_start(bk_sb[:], bk_d.ap())
        bv_sb = singles.tile([1, 128], F32)
        nc.sync.dma_start(bv_sb[:], bv_d.ap())
        bo_sb = singles.tile([1, 64], F32)
        nc.sync.dma_start(bo_sb[:], bo_d.ap())
        gam_sb = singles.tile([128, 1], F32)
        nc.sync.dma_start(gam_sb[:], gam_d.ap())
        bet_sb = singles.tile([128, 1], F32)
        nc.sync.dma_start(bet_sb[:], bet_d.ap())
        comb_sb = singles.tile([128, 128], F32)
        nc.sync.dma_start(comb_sb[:], comb_d.ap())

        ones_sb = singles.tile([128, 512], F32)
        nc.vector.memset(ones_sb[:], 1.0)

        q_dup = singles.tile([128, HALF], F32)
        kt_sb = singles.tile([128, HALF], F32)
        v_all = singles.tile([128, 65 * 32], F32)
        attnexp = singles.tile([128, 512 * 32], F32)
        out_sb = singles.tile([64, HALF], F32)

        # ones column of v_all (chunk t's column 64 -> softmax denominator)
        v3 = v_all[:].rearrange("p (t e) -> p t e", e=65)
        nc.vector.memset(v3[:, :, 64:65], 1.0)

        # ---- GroupNorm (stats per partition per 512-slice, then a
        # block-diagonal averaging matmul combines across channels) ----
        smat = stats.tile([128, 8], F32)  # cols 0-3 mean, 4-7 E[x^2]
        for r in range(4):
            st6 = stats.tile([128, 6], F32, tag="st6")
            nc.vector.bn_stats(st6[:], x_sb[:, 512 * r : 512 * r + 512])
            mv = stats.tile([128, 2], F32, tag="mv")
            nc.vector.bn_aggr(mv[:], st6[:])
            nc.vector.tensor_copy(smat[:, r : r + 1], mv[:, 0:1])
            sq = stats.tile([128, 1], F32, tag="sq")
            nc.vector.tensor_mul(sq[:], mv[:, 0:1], mv[:, 0:1])
            nc.vector.tensor_add(smat[:, 4 + r : 5 + r], mv[:, 1:2], sq[:])

        with tc.tile_pool(name="pre_ps", bufs=2, space="PSUM") as pre_ps:
            cps = pre_ps.tile([128, 8], F32, tag="pre")
            nc.tensor.matmul(cps[:], comb_sb[:], smat[:], start=True, stop=True)
            gstat = stats.tile([128, 8], F32)  # 0-3 mean_g, 4-7 E2_g
            nc.vector.tensor_copy(gstat[:], cps[:])

            var_g = stats.tile([128, 4], F32)
            nc.vector.tensor_mul(var_g[:], gstat[:, 0:4], gstat[:, 0:4])
            nc.vector.tensor_tensor(
                var_g[:], gstat[:, 4:8], var_g[:], op=mybir.AluOpType.subtract
            )
            # rstd = exp(-0.5 * ln(var + eps)); Ln/Exp share one ACT table set
            eps_sb = stats.tile([128, 1], F32)
            nc.vector.memset(eps_sb[:], EPS)
            rstd = stats.tile([128, 4], F32)
            nc.scalar.activation(
                rstd[:], var_g[:], mybir.ActivationFunctionType.Ln, bias=eps_sb[:]
            )
            nc.scalar.activation(
                rstd[:], rstd[:], mybir.ActivationFunctionType.Exp, scale=-0.5
            )
            gsc = stats.tile([128, 4], F32)
            nc.vector.tensor_scalar_mul(gsc[:], rstd[:], gam_sb[:])
            gbias = stats.tile([128, 4], F32)
            nc.vector.tensor_mul(gbias[:], gstat[:, 0:4], gsc[:])
            nc.vector.tensor_scalar(
                out=gbias[:], in0=gbias[:], scalar1=-1.0, scalar2=bet_sb[:],
                op0=mybir.AluOpType.mult, op1=mybir.AluOpType.add,
            )
            # xn = x * gsc + gbias  (in place)
            for r in range(4):
                nc.vector.tensor_scalar(
                    out=x_sb[:, 512 * r : 512 * r + 512],
                    in0=x_sb[:, 512 * r : 512 * r + 512],
                    scalar1=gsc[:, r : r + 1], scalar2=gbias[:, r : r + 1],
                    op0=mybir.AluOpType.mult, op1=mybir.AluOpType.add,
                )

            # ---- Q/K/V projections ----
            # q^T duplicated on both partition halves: lhsT = [Wq | Wq]
            for t in range(4):
                sl = slice(512 * t, 512 * t + 512)
                ps = pre_ps.tile([128, 512], F32, tag="pre")
                nc.tensor.matmul(
                    ps[:], _mm(bq_sb[:]), _mm(ones_sb[0:1, :]), start=True,
                    stop=False
                )
                nc.tensor.matmul(
                    ps[:], _mm(wq_sb[:]), _mm(x_sb[0:64, sl]), start=False,
                    stop=True
                )
                nc.vector.tensor_copy(q_dup[:, sl], ps[:])
            # k^T packed by half: lhsT = blockdiag(Wk, Wk)
            for t in range(4):
                sl = slice(512 * t, 512 * t + 512)
                ps = pre_ps.tile([128, 512], F32, tag="pre")
                nc.tensor.matmul(
                    ps[:], _mm(bk_sb[:]), _mm(ones_sb[0:1, :]), start=True,
                    stop=False
                )
                nc.tensor.matmul(
                    ps[:], _mm(wk_sb[:]), _mm(x_sb[:, sl]), start=False,
                    stop=True
                )
                nc.vector.tensor_copy(kt_sb[:, sl], ps[:])
            # v position-major, two chunks per matmul: out[pos, (half, c)]
            for u in range(16):
                sl = slice(128 * u, 128 * u + 128)
                ps = pre_ps.tile([128, 128], F32, tag="pre")
                nc.tensor.matmul(
                    ps[:], _mm(ones_sb[0:1, 0:128]), _mm(bv_sb[:]), start=True,
                    stop=False
                )
                nc.tensor.matmul(
                    ps[:], _mm(x_sb[:, sl]), _mm(wv_sb[:]), start=False,
                    stop=True
                )
                nc.vector.tensor_copy(v_all[:, 65 * u : 65 * u + 64], ps[:, 0:64])
                nc.vector.tensor_copy(
                    v_all[:, 65 * (u + 16) : 65 * (u + 16) + 64], ps[:, 64:128]
                )

        with tc.tile_pool(name="sc_ps", bufs=2, space="PSUM") as sc_ps, \
             tc.tile_pool(name="pj_ps", bufs=2, space="PSUM") as pj_ps, \
             tc.tile_pool(name="work", bufs=2) as work:

            batches = [list(range(t0, min(t0 + EXP_BATCH, 32)))
                       for t0 in range(0, 32, EXP_BATCH)]

            # ---- attention, one 512-query tile at a time ----
            for n in range(4):
                qsl = slice(512 * n, 512 * n + 512)
                # scores^T + exp, chunk t = 128 kv positions
                for batch in batches:
                    nb = len(batch)
                    ps = sc_ps.tile([128, 512 * EXP_BATCH], F32, tag="sc")
                    for i, t in enumerate(batch):
                        j = t // 16
                        lhsT = kt_sb[64 * j : 64 * j + 64,
                                     128 * (t % 16) : 128 * (t % 16) + 128]
                        rhs = q_dup[64 * j : 64 * j + 64, qsl]
                        nc.tensor.matmul(
                            ps[:, 512 * i : 512 * i + 512], _mm(lhsT), _mm(rhs),
                            start=True, stop=True,
                        )
                    off = 512 * batch[0]
                    nc.scalar.activation(
                        attnexp[:, off : off + 512 * nb], ps[:, 0 : 512 * nb],
                        mybir.ActivationFunctionType.Exp, scale=SCALE,
                    )
                # attn^T @ [V | 1]: accumulate over the 32 kv chunks
                pacc = pj_ps.tile([65, 512], F32, tag="pj")
                for t in range(32):
                    nc.tensor.matmul(
                        pacc[:], _mm(v_all[:, 65 * t : 65 * t + 65]),
                        _mm(attnexp[:, 512 * t : 512 * t + 512]),
                        start=(t == 0), stop=(t == 31),
                    )
                # normalize: proj^T = pacc[0:64] * (1/denom) broadcast
                rd = work.tile([65, 512], F32, tag="rd")
                nc.vector.reciprocal(rd[64:65, :], pacc[64:65, :])
                bc_ps = pj_ps.tile([64, 512], F32, tag="pj")
                nc.tensor.matmul(
                    bc_ps[:], ones_sb[64:65, 0:64], rd[64:65, :], start=True,
                    stop=True
                )
                bc_sb = work.tile([64, 512], F32, tag="bc")
                nc.vector.tensor_copy(bc_sb[:], bc_ps[:])
                projn = work.tile([64, 512], F32, tag="projn")
                nc.vector.tensor_mul(projn[:], pacc[0:64, :], bc_sb[:])
                # out-projection + bias + residual
                fps = pj_ps.tile([64, 512], F32, tag="pj")
                nc.tensor.matmul(
                    fps[:], _mm(bo_sb[:]), _mm(ones_sb[0:1, :]), start=True,
                    stop=False
                )
                nc.tensor.matmul(
                    fps[:], _mm(wo_sb[:]), _mm(projn[:]), start=False, stop=True
                )
                nc.vector.tensor_add(out_sb[:, qsl], fps[:], x_sb[0:64, qsl])

        nc.sync.dma_start(out_d.ap(), out_sb[:])

    nc.compile()
    return nc


def host_prep(x, gamma, beta, Wq, bq, Wk, bk, Wv, bv, Wo, bo):
    """Build the 8 per-core input dicts."""
    f32 = lambda a: np.ascontiguousarray(np.asarray(a, np.float32))
    x = f32(x)
    gamma, beta = f32(gamma), f32(beta)
    Wq, Wk, Wv, Wo = f32(Wq), f32(Wk), f32(Wv), f32(Wo)
    bq, bk, bv, bo = f32(bq), f32(bk), f32(bv), f32(bo)

    wq_dup = np.ascontiguousarray(np.concatenate([Wq, Wq], axis=1))
    z = np.zeros((64, 64), np.float32)
    wk_blk = np.ascontiguousarray(np.block([[Wk, z], [z, Wk]]))
    wv_blk = np.ascontiguousarray(np.block([[Wv, z], [z, Wv]]))
    comb = np.zeros((128, 128), np.float32)
    comb[:64, :64] = 1.0 / 64.0
    comb[64:, 64:] = 1.0 / 64.0
    shared = {
        "wq": wq_dup, "wk": wk_blk, "wv": wv_blk, "wo": Wo,
        "bq": np.ascontiguousarray(np.tile(bq, 2)[None]),
        "bk": np.ascontiguousarray(np.tile(bk, 2)[None]),
        "bv": np.ascontiguousarray(np.tile(bv, 2)[None]),
        "bo": np.ascontiguousarray(bo[None]),
        "gam": np.ascontiguousarray(np.tile(gamma, 2)[:, None]),
        "bet": np.ascontiguousarray(np.tile(beta, 2)[:, None]),
        "comb": comb,
    }
    in_maps = []
    for core in range(8):
        b, h = core // 2, core % 2
        xT = x[b].reshape(HW, C).T  # [64, 4096]
        halves = xT.reshape(C, 2, HALF)[:, [h, 1 - h], :]
        xp = np.ascontiguousarray(halves.transpose(1, 0, 2).reshape(128, HALF))
        in_maps.append({"xp": xp, **shared})
    return in_maps


def assemble(results, dtype):
    out = np.empty((B, HW, C), np.float32)
    for core in range(8):
        b, h = core // 2, core % 2
        out[b, HALF * h : HALF * h + HALF] = results[core]["out"].T
    return out.reshape(B, H, W, C).astype(dtype, copy=False)


_NC_CACHE = []


def kernel(x, gamma, beta, Wq, bq, Wk, bk, Wv, bv, Wo, bo):
    from concourse.bass_utils import run_bass_kernel_spmd

    if not _NC_CACHE:
        _NC_CACHE.append(build_nc())
    nc = _NC_CACHE[0]
    in_maps = host_prep(x, gamma, beta, Wq, bq, Wk, bk, Wv, bv, Wo, bo)
    res = run_bass_kernel_spmd(nc, in_maps, core_ids=list(range(8)))
    return assemble(res.results, np.asarray(x).dtype)


if __name__ == "__main__":
    rng = np.random.default_rng(0)
    inputs = {
        "x": rng.standard_normal((B, H, W, C)).astype(np.float32),
        "gamma": np.ones(C, np.float32), "beta": np.zeros(C, np.float32),
        "Wq": (rng.standard_normal((C, C)) / 8).astype(np.float32),
        "bq": np.zeros(C, np.float32),
        "Wk": (rng.standard_normal((C, C)) / 8).astype(np.float32),
        "bk": np.zeros(C, np.float32),
        "Wv": (rng.standard_normal((C, C)) / 8).astype(np.float32),
        "bv": np.zeros(C, np.float32),
        "Wo": (rng.standard_normal((C, C)) / 8).astype(np.float32),
        "bo": np.zeros(C, np.float32),
    }
    out = kernel(**inputs)
    print("kernel ran, out shape", out.shape, out.dtype)


# revision 7
# speedup vs baseline: 1.8906x; 1.8906x over previous
"""Trainium2 Bass kernel for nn_AttentionBlock (B=4, H=W=64, C=64, GroupNorm(8) +
full spatial self-attention), distributed over 8 NeuronCores.

Sharding: core i handles batch b=i//2 and query-half h=i%2 (2048 of the 4096
spatial positions). Each core computes the full GroupNorm and K/V for its
image (cheap) and attention only for its query half. No collectives.

Device layout: channel-on-partition ("xT") layout, with the two position
halves of an image packed onto partitions [(half, channel)] -> 128 partitions.
Scores are computed transposed (positions' on partitions) so that
- softmax reduction over positions' is a PE matmul (ones column in V), and
- the attention @ V contraction needs no transposes at all.
exp() runs on ScalarE reading PSUM and writing SBUF directly.

PE matmuls run in float32r (1 cycle/row at N>=256, ~tf32 mantissa); GroupNorm
statistics and the residual path stay full fp32.
"""

import sys

sys.path.insert(0, "/opt/trn_rl_repo")

import numpy as np

import concourse.bacc as bacc
import concourse.tile as tile
from concourse import mybir

B, H, W, C = 4, 64, 64, 64
HW = H * W  # 4096
HALF = HW // 2  # 2048
EPS = 1e-5
SCALE = C ** -0.5  # folded into exp()

F32 = mybir.dt.float32
# dtype of the PE matmul operands: float32r runs at 4x the float32 rate.
MDT = mybir.dt.float32r

EXP_BATCH = 3  # pos'-chunks (PSUM banks) per exp() call


def build_nc():
    nc = bacc.Bacc("TRN2", debug=False, num_devices=8)

    # ---- DRAM I/O ----
    xp_d = nc.dram_tensor("xp", [128, HALF], F32, kind="ExternalInput")
    wq_d = nc.dram_tensor("wq", [64, 128], MDT, kind="ExternalInput")
    wk_d = nc.dram_tensor("wk", [128, 128], MDT, kind="ExternalInput")
    wv_d = nc.dram_tensor("wv", [128, 128], MDT, kind="ExternalInput")
    wo_d = nc.dram_tensor("wo", [64, 64], MDT, kind="ExternalInput")
    bq_d = nc.dram_tensor("bq", [1, 128], MDT, kind="ExternalInput")
    bk_d = nc.dram_tensor("bk", [1, 128], MDT, kind="ExternalInput")
    bv_d = nc.dram_tensor("bv", [1, 128], MDT, kind="ExternalInput")
    bo_d = nc.dram_tensor("bo", [1, 64], MDT, kind="ExternalInput")
    gam_d = nc.dram_tensor("gam", [128, 1], F32, kind="ExternalInput")
    bet_d = nc.dram_tensor("bet", [128, 1], F32, kind="ExternalInput")
    comb_d = nc.dram_tensor("comb", [128, 128], F32, kind="ExternalInput")
    out_d = nc.dram_tensor("out", [64, HALF], F32, kind="ExternalOutput")

    with tile.TileContext(nc) as tc, \
         tc.tile_pool(name="singles", bufs=1) as singles, \
         tc.tile_pool(name="stats", bufs=4) as stats:

        # ---- constant loads ----
        x_sb = singles.tile([128, HALF], F32)
        nc.sync.dma_start(x_sb[:], xp_d.ap())
        wq_sb = singles.tile([64, 128], MDT)
        nc.sync.dma_start(wq_sb[:], wq_d.ap())
        wk_sb = singles.tile([128, 128], MDT)
        nc.sync.dma_start(wk_sb[:], wk_d.ap())
        wv_sb = singles.tile([128, 128], MDT)
        nc.sync.dma_start(wv_sb[:], wv_d.ap())
        wo_sb = singles.tile([64, 64], MDT)
        nc.sync.dma_start(wo_sb[:], wo_d.ap())
        bq_sb = singles.tile([1, 128], MDT)
        nc.sync.dma_start(bq_sb[:], bq_d.ap())
        bk_sb = singles.tile([1, 128], MDT)
        nc.sync.dma_start(bk_sb[:], bk_d.ap())
        bv_sb = singles.tile([1, 128], MDT)
        nc.sync.dma_start(bv_sb[:], bv_d.ap())
        bo_sb = singles.tile([1, 64], MDT)
        nc.sync.dma_start(bo_sb[:], bo_d.ap())
        gam_sb = singles.tile([128, 1], F32)
        nc.sync.dma_start(gam_sb[:], gam_d.ap())
        bet_sb = singles.tile([128, 1], F32)
        nc.sync.dma_start(bet_sb[:], bet_d.ap())
        comb_sb = singles.tile([128, 128], F32)
        nc.sync.dma_start(comb_sb[:], comb_d.ap())

        ones_f = singles.tile([128, 512], F32)
        nc.vector.memset(ones_f[:], 1.0)
        ones_sb = singles.tile([128, 512], MDT)
        nc.vector.tensor_copy(ones_sb[:], ones_f[:])

        xn_r = singles.tile([128, HALF], MDT)
        q_dup = singles.tile([128, HALF], MDT)
        kt_sb = singles.tile([128, HALF], MDT)
        v_all = singles.tile([128, 65 * 32], MDT)
        attnexp = singles.tile([128, 512 * 32], MDT)
        out_sb = singles.tile([64, HALF], F32)

        # ones column of v_all (chunk t's column 64 -> softmax denominator)
        v3 = v_all[:].rearrange("p (t e) -> p t e", e=65)
        nc.vector.tensor_copy(
            v3[:, :, 64:65], ones_f[:, 0:32].rearrange("p (a b) -> p a b", b=1)
        )

        # ---- GroupNorm (stats per partition per 512-slice, then a
        # block-diagonal averaging matmul combines across channels) ----
        smat = stats.tile([128, 8], F32)  # cols 0-3 mean, 4-7 E[x^2]
        for r in range(4):
            st6 = stats.tile([128, 6], F32, tag="st6")
            nc.vector.bn_stats(st6[:], x_sb[:, 512 * r : 512 * r + 512])
            mv = stats.tile([128, 2], F32, tag="mv")
            nc.vector.bn_aggr(mv[:], st6[:])
            nc.vector.tensor_copy(smat[:, r : r + 1], mv[:, 0:1])
            sq = stats.tile([128, 1], F32, tag="sq")
            nc.vector.tensor_mul(sq[:], mv[:, 0:1], mv[:, 0:1])
            nc.vector.tensor_add(smat[:, 4 + r : 5 + r], mv[:, 1:2], sq[:])

        with tc.tile_pool(name="pre_ps", bufs=2, space="PSUM") as pre_ps:
            cps = pre_ps.tile([128, 8], F32, tag="pre")
            nc.tensor.matmul(cps[:], comb_sb[:], smat[:], start=True, stop=True)
            gstat = stats.tile([128, 8], F32)  # 0-3 mean_g, 4-7 E2_g
            nc.vector.tensor_copy(gstat[:], cps[:])

            var_g = stats.tile([128, 4], F32)
            nc.vector.tensor_mul(var_g[:], gstat[:, 0:4], gstat[:, 0:4])
            nc.vector.tensor_tensor(
                var_g[:], gstat[:, 4:8], var_g[:], op=mybir.AluOpType.subtract
            )
            # rstd = exp(-0.5 * ln(var + eps)); Ln/Exp share one ACT table set
            eps_sb = stats.tile([128, 1], F32)
            nc.vector.memset(eps_sb[:], EPS)
            rstd = stats.tile([128, 4], F32)
            nc.scalar.activation(
                rstd[:], var_g[:], mybir.ActivationFunctionType.Ln, bias=eps_sb[:]
            )
            nc.scalar.activation(
                rstd[:], rstd[:], mybir.ActivationFunctionType.Exp, scale=-0.5
            )
            gsc = stats.tile([128, 4], F32)
            nc.vector.tensor_scalar_mul(gsc[:], rstd[:], gam_sb[:])
            gbias = stats.tile([128, 4], F32)
            nc.vector.tensor_mul(gbias[:], gstat[:, 0:4], gsc[:])
            nc.vector.tensor_scalar(
                out=gbias[:], in0=gbias[:], scalar1=-1.0, scalar2=bet_sb[:],
                op0=mybir.AluOpType.mult, op1=mybir.AluOpType.add,
            )
            # xn = x * gsc + gbias: fp32 in place (residual path) and rounded
            # f32r copy (matmul path)
            for r in range(4):
                nc.vector.tensor_scalar(
                    out=x_sb[:, 512 * r : 512 * r + 512],
                    in0=x_sb[:, 512 * r : 512 * r + 512],
                    scalar1=gsc[:, r : r + 1], scalar2=gbias[:, r : r + 1],
                    op0=mybir.AluOpType.mult, op1=mybir.AluOpType.add,
                )
            nc.vector.tensor_copy(xn_r[:], x_sb[:])

            # ---- Q/K/V projections ----
            # q^T duplicated on both partition halves: lhsT = [Wq | Wq]
            for t in range(4):
                sl = slice(512 * t, 512 * t + 512)
                ps = pre_ps.tile([128, 512], F32, tag="pre")
                nc.tensor.matmul(
                    ps[:], bq_sb[:], ones_sb[0:1, :], start=True, stop=False
                )
                nc.tensor.matmul(
                    ps[:], wq_sb[:], xn_r[0:64, sl], start=False, stop=True
                )
                nc.vector.tensor_copy(q_dup[:, sl], ps[:])
            # k^T packed by half: lhsT = blockdiag(Wk, Wk)
            for t in range(4):
                sl = slice(512 * t, 512 * t + 512)
                ps = pre_ps.tile([128, 512], F32, tag="pre")
                nc.tensor.matmul(
                    ps[:], bk_sb[:], ones_sb[0:1, :], start=True, stop=False
                )
                nc.tensor.matmul(
                    ps[:], wk_sb[:], xn_r[:, sl], start=False, stop=True
                )
                nc.vector.tensor_copy(kt_sb[:, sl], ps[:])
            # v position-major, two chunks per matmul: out[pos, (half, c)]
            for u in range(16):
                sl = slice(128 * u, 128 * u + 128)
                ps = pre_ps.tile([128, 128], F32, tag="pre")
                nc.tensor.matmul(
                    ps[:], ones_sb[0:1, 0:128], bv_sb[:], start=True, stop=False
                )
                nc.tensor.matmul(
                    ps[:], xn_r[:, sl], wv_sb[:], start=False, stop=True
                )
                nc.vector.tensor_copy(v_all[:, 65 * u : 65 * u + 64], ps[:, 0:64])
                nc.vector.tensor_copy(
                    v_all[:, 65 * (u + 16) : 65 * (u + 16) + 64], ps[:, 64:128]
                )

        with tc.tile_pool(name="sc_ps", bufs=2, space="PSUM") as sc_ps, \
             tc.tile_pool(name="pj_ps", bufs=2, space="PSUM") as pj_ps, \
             tc.tile_pool(name="work", bufs=2) as work:

            batches = [list(range(t0, min(t0 + EXP_BATCH, 32)))
                       for t0 in range(0, 32, EXP_BATCH)]

            # ---- attention, one 512-query tile at a time ----
            for n in range(4):
                qsl = slice(512 * n, 512 * n + 512)
                # scores^T + exp, chunk t = 128 kv positions
                for batch in batches:
                    nb = len(batch)
                    ps = sc_ps.tile([128, 512 * EXP_BATCH], F32, tag="sc")
                    for i, t in enumerate(batch):
                        j = t // 16
                        lhsT = kt_sb[64 * j : 64 * j + 64,
                                     128 * (t % 16) : 128 * (t % 16) + 128]
                        rhs = q_dup[64 * j : 64 * j + 64, qsl]
                        nc.tensor.matmul(
                            ps[:, 512 * i : 512 * i + 512], lhsT, rhs,
                            start=True, stop=True,
                        )
                    off = 512 * batch[0]
                    nc.scalar.activation(
                        attnexp[:, off : off + 512 * nb], ps[:, 0 : 512 * nb],
                        mybir.ActivationFunctionType.Exp, scale=SCALE,
                    )
                # attn^T @ [V | 1]: accumulate over the 32 kv chunks
                pacc = pj_ps.tile([65, 512], F32, tag="pj")
                for t in range(32):
                    nc.tensor.matmul(
                        pacc[:], v_all[:, 65 * t : 65 * t + 65],
                        attnexp[:, 512 * t : 512 * t + 512],
                        start=(t == 0), stop=(t == 31),
                    )
                # normalize: proj^T = pacc[0:64] * (1/denom) broadcast
                rd = work.tile([65, 512], MDT, tag="rd")
                with nc.allow_low_precision(reason="softmax denom in f32r"):
                    nc.vector.reciprocal(rd[64:65, :], pacc[64:65, :])
                bc_ps = pj_ps.tile([64, 512], F32, tag="pj")
                nc.tensor.matmul(
                    bc_ps[:], ones_sb[64:65, 0:64], rd[64:65, :], start=True,
                    stop=True
                )
                bc_sb = work.tile([64, 512], F32, tag="bc")
                nc.vector.tensor_copy(bc_sb[:], bc_ps[:])
                projn = work.tile([64, 512], MDT, tag="projn")
                nc.vector.tensor_mul(projn[:], pacc[0:64, :], bc_sb[:])
                # out-projection + bias + residual
                fps = pj_ps.tile([64, 512], F32, tag="pj")
                nc.tensor.matmul(
                    fps[:], bo_sb[:], ones_sb[0:1, :], start=True, stop=False
                )
                nc.tensor.matmul(
                    fps[:], wo_sb[:], projn[:], start=False, stop=True
                )
                nc.vector.tensor_add(out_sb[:, qsl], fps[:], x_sb[0:64, qsl])

        nc.sync.dma_start(out_d.ap(), out_sb[:])

    nc.compile()
    return nc


def host_prep(x, gamma, beta, Wq, bq, Wk, bk, Wv, bv, Wo, bo):
    """Build the 8 per-core input dicts."""
    f32 = lambda a: np.ascontiguousarray(np.asarray(a, np.float32))
    x = f32(x)
    gamma, beta = f32(gamma), f32(beta)
    Wq, Wk, Wv, Wo = f32(Wq), f32(Wk), f32(Wv), f32(Wo)
    bq, bk, bv, bo = f32(bq), f32(bk), f32(bv), f32(bo)

    wq_dup = np.ascontiguousarray(np.concatenate([Wq, Wq], axis=1))
    z = np.zeros((64, 64), np.float32)
    wk_blk = np.ascontiguousarray(np.block([[Wk, z], [z, Wk]]))
    wv_blk = np.ascontiguousarray(np.block([[Wv, z], [z, Wv]]))
    comb = np.zeros((128, 128), np.float32)
    comb[:64, :64] = 1.0 / 64.0
    comb[64:, 64:] = 1.0 / 64.0
    shared = {
        "wq": wq_dup, "wk": wk_blk, "wv": wv_blk, "wo": Wo,
        "bq": np.ascontiguousarray(np.tile(bq, 2)[None]),
        "bk": np.ascontiguousarray(np.tile(bk, 2)[None]),
        "bv": np.ascontiguousarray(np.tile(bv, 2)[None]),
        "bo": np.ascontiguousarray(bo[None]),
        "gam": np.ascontiguousarray(np.tile(gamma, 2)[:, None]),
        "bet": np.ascontiguousarray(np.tile(beta, 2)[:, None]),
        "comb": comb,
    }
    in_maps = []
    for core in range(8):
        b, h = core // 2, core % 2
        xT = x[b].reshape(HW, C).T  # [64, 4096]
        halves = xT.reshape(C, 2, HALF)[:, [h, 1 - h], :]
        xp = np.ascontiguousarray(halves.transpose(1, 0, 2).reshape(128, HALF))
        in_maps.append({"xp": xp, **shared})
    return in_maps


def assemble(results, dtype):
    out = np.empty((B, HW, C), np.float32)
    for core in range(8):
        b, h = core // 2, core % 2
        out[b, HALF * h : HALF * h + HALF] = results[core]["out"].T
    return out.reshape(B, H, W, C).astype(dtype, copy=False)


_NC_CACHE = []


def kernel(x, gamma, beta, Wq, bq, Wk, bk, Wv, bv, Wo, bo):
    from concourse.bass_utils import run_bass_kernel_spmd

    if not _NC_CACHE:
        _NC_CACHE.append(build_nc())
    nc = _NC_CACHE[0]
    in_maps = host_prep(x, gamma, beta, Wq, bq, Wk, bk, Wv, bv, Wo, bo)
    res = run_bass_kernel_spmd(nc, in_maps, core_ids=list(range(8)))
    return assemble(res.results, np.asarray(x).dtype)


if __name__ == "__main__":
    rng = np.random.default_rng(0)
    inputs = {
        "x": rng.standard_normal((B, H, W, C)).astype(np.float32),
        "gamma": np.ones(C, np.float32), "beta": np.zeros(C, np.float32),
        "Wq": (rng.standard_normal((C, C)) / 8).astype(np.float32),
        "bq": np.zeros(C, np.float32),
        "Wk": (rng.standard_normal((C, C)) / 8).astype(np.float32),
        "bk": np.zeros(C, np.float32),
        "Wv": (rng.standard_normal((C, C)) / 8).astype(np.float32),
        "bv": np.zeros(C, np.float32),
        "Wo": (rng.standard_normal((C, C)) / 8).astype(np.float32),
        "bo": np.zeros(C, np.float32),
    }
    out = kernel(**inputs)
    print("kernel ran, out shape", out.shape, out.dtype)


# revision 8
# speedup vs baseline: 2.2401x; 1.1849x over previous
"""Trainium2 Bass kernel for nn_AttentionBlock (B=4, H=W=64, C=64, GroupNorm(8) +
full spatial self-attention), distributed over 8 NeuronCores.

Sharding: core i handles batch b=i//2 and query-half h=i%2 (2048 of the 4096
spatial positions). Each core computes the full GroupNorm and K/V for its
image (cheap) and attention only for its query half. No collectives.

Device layout: channel-on-partition ("xT") layout, with the two position
halves of an image packed onto partitions [(half, channel)] -> 128 partitions.
Scores are computed transposed (positions' on partitions) so that
- softmax reduction over positions' is a PE matmul (ones column in V), and
- the attention @ V contraction needs no transposes at all.
exp() runs on ScalarE reading PSUM and writing SBUF directly.

PE matmuls run in float32r (1 cycle/row at N>=256, ~tf32 mantissa); GroupNorm
statistics and the residual path stay full fp32.
"""

import sys

sys.path.insert(0, "/opt/trn_rl_repo")

import numpy as np

import concourse.bacc as bacc
import concourse.tile as tile
from concourse import mybir

B, H, W, C = 4, 64, 64, 64
HW = H * W  # 4096
HALF = HW // 2  # 2048
EPS = 1e-5
SCALE = C ** -0.5  # folded into exp()

F32 = mybir.dt.float32
# dtype of the PE matmul operands: float32r runs at 4x the float32 rate.
MDT = mybir.dt.bfloat16

EXP_BATCH = 3  # pos'-chunks (PSUM banks) per exp() call


def build_nc():
    nc = bacc.Bacc("TRN2", debug=False, num_devices=8)

    # ---- DRAM I/O ----
    xp_d = nc.dram_tensor("xp", [128, HALF], F32, kind="ExternalInput")
    wq_d = nc.dram_tensor("wq", [64, 128], MDT, kind="ExternalInput")
    wk_d = nc.dram_tensor("wk", [128, 128], MDT, kind="ExternalInput")
    wv_d = nc.dram_tensor("wv", [128, 128], MDT, kind="ExternalInput")
    wo_d = nc.dram_tensor("wo", [64, 64], MDT, kind="ExternalInput")
    bq_d = nc.dram_tensor("bq", [1, 128], MDT, kind="ExternalInput")
    bk_d = nc.dram_tensor("bk", [1, 128], MDT, kind="ExternalInput")
    bv_d = nc.dram_tensor("bv", [1, 128], MDT, kind="ExternalInput")
    bo_d = nc.dram_tensor("bo", [1, 64], MDT, kind="ExternalInput")
    gam_d = nc.dram_tensor("gam", [128, 1], F32, kind="ExternalInput")
    bet_d = nc.dram_tensor("bet", [128, 1], F32, kind="ExternalInput")
    comb_d = nc.dram_tensor("comb", [128, 128], F32, kind="ExternalInput")
    out_d = nc.dram_tensor("out", [64, HALF], F32, kind="ExternalOutput")

    with tile.TileContext(nc) as tc, \
         tc.tile_pool(name="singles", bufs=1) as singles, \
         tc.tile_pool(name="stats", bufs=4) as stats:

        # ---- constant loads ----
        x_sb = singles.tile([128, HALF], F32)
        nc.sync.dma_start(x_sb[:], xp_d.ap())
        wq_sb = singles.tile([64, 128], MDT)
        nc.sync.dma_start(wq_sb[:], wq_d.ap())
        wk_sb = singles.tile([128, 128], MDT)
        nc.sync.dma_start(wk_sb[:], wk_d.ap())
        wv_sb = singles.tile([128, 128], MDT)
        nc.sync.dma_start(wv_sb[:], wv_d.ap())
        wo_sb = singles.tile([64, 64], MDT)
        nc.sync.dma_start(wo_sb[:], wo_d.ap())
        bq_sb = singles.tile([1, 128], MDT)
        nc.sync.dma_start(bq_sb[:], bq_d.ap())
        bk_sb = singles.tile([1, 128], MDT)
        nc.sync.dma_start(bk_sb[:], bk_d.ap())
        bv_sb = singles.tile([1, 128], MDT)
        nc.sync.dma_start(bv_sb[:], bv_d.ap())
        bo_sb = singles.tile([1, 64], MDT)
        nc.sync.dma_start(bo_sb[:], bo_d.ap())
        gam_sb = singles.tile([128, 1], F32)
        nc.sync.dma_start(gam_sb[:], gam_d.ap())
        bet_sb = singles.tile([128, 1], F32)
        nc.sync.dma_start(bet_sb[:], bet_d.ap())
        comb_sb = singles.tile([128, 128], F32)
        nc.sync.dma_start(comb_sb[:], comb_d.ap())

        ones_f = singles.tile([128, 512], F32)
        nc.vector.memset(ones_f[:], 1.0)
        ones_sb = singles.tile([128, 512], MDT)
        nc.vector.tensor_copy(ones_sb[:], ones_f[:])

        xn_r = singles.tile([128, HALF], MDT)
        q_dup = singles.tile([128, HALF], MDT)
        kt_sb = singles.tile([128, HALF], MDT)
        v_all = singles.tile([128, 65 * 32], MDT)
        attnexp = singles.tile([128, 512 * 32], MDT)
        out_sb = singles.tile([64, HALF], F32)

        # ones column of v_all (chunk t's column 64 -> softmax denominator)
        v3 = v_all[:].rearrange("p (t e) -> p t e", e=65)
        nc.vector.tensor_copy(
            v3[:, :, 64:65], ones_f[:, 0:32].rearrange("p (a b) -> p a b", b=1)
        )

        # ---- GroupNorm (stats per partition per 512-slice, then a
        # block-diagonal averaging matmul combines across channels) ----
        smat = stats.tile([128, 8], F32)  # cols 0-3 mean, 4-7 E[x^2]
        for r in range(4):
            st6 = stats.tile([128, 6], F32, tag="st6")
            nc.vector.bn_stats(st6[:], x_sb[:, 512 * r : 512 * r + 512])
            mv = stats.tile([128, 2], F32, tag="mv")
            nc.vector.bn_aggr(mv[:], st6[:])
            nc.vector.tensor_copy(smat[:, r : r + 1], mv[:, 0:1])
            sq = stats.tile([128, 1], F32, tag="sq")
            nc.vector.tensor_mul(sq[:], mv[:, 0:1], mv[:, 0:1])
            nc.vector.tensor_add(smat[:, 4 + r : 5 + r], mv[:, 1:2], sq[:])

        with tc.tile_pool(name="pre_ps", bufs=2, space="PSUM") as pre_ps:
            cps = pre_ps.tile([128, 8], F32, tag="pre")
            nc.tensor.matmul(cps[:], comb_sb[:], smat[:], start=True, stop=True)
            gstat = stats.tile([128, 8], F32)  # 0-3 mean_g, 4-7 E2_g
            nc.vector.tensor_copy(gstat[:], cps[:])

            var_g = stats.tile([128, 4], F32)
            nc.vector.tensor_mul(var_g[:], gstat[:, 0:4], gstat[:, 0:4])
            nc.vector.tensor_tensor(
                var_g[:], gstat[:, 4:8], var_g[:], op=mybir.AluOpType.subtract
            )
            # rstd = exp(-0.5 * ln(var + eps)); Ln/Exp share one ACT table set
            eps_sb = stats.tile([128, 1], F32)
            nc.vector.memset(eps_sb[:], EPS)
            rstd = stats.tile([128, 4], F32)
            nc.scalar.activation(
                rstd[:], var_g[:], mybir.ActivationFunctionType.Ln, bias=eps_sb[:]
            )
            nc.scalar.activation(
                rstd[:], rstd[:], mybir.ActivationFunctionType.Exp, scale=-0.5
            )
            gsc = stats.tile([128, 4], F32)
            nc.vector.tensor_scalar_mul(gsc[:], rstd[:], gam_sb[:])
            gbias = stats.tile([128, 4], F32)
            nc.vector.tensor_mul(gbias[:], gstat[:, 0:4], gsc[:])
            nc.vector.tensor_scalar(
                out=gbias[:], in0=gbias[:], scalar1=-1.0, scalar2=bet_sb[:],
                op0=mybir.AluOpType.mult, op1=mybir.AluOpType.add,
            )
            # xn = x * gsc + gbias: fp32 in place (residual path) and rounded
            # f32r copy (matmul path)
            for r in range(4):
                nc.vector.tensor_scalar(
                    out=x_sb[:, 512 * r : 512 * r + 512],
                    in0=x_sb[:, 512 * r : 512 * r + 512],
                    scalar1=gsc[:, r : r + 1], scalar2=gbias[:, r : r + 1],
                    op0=mybir.AluOpType.mult, op1=mybir.AluOpType.add,
                )
            nc.vector.tensor_copy(xn_r[:], x_sb[:])

            # ---- Q/K/V projections ----
            # q^T duplicated on both partition halves: lhsT = [Wq | Wq]
            for t in range(4):
                sl = slice(512 * t, 512 * t + 512)
                ps = pre_ps.tile([128, 512], F32, tag="pre")
                nc.tensor.matmul(
                    ps[:], bq_sb[:], ones_sb[0:1, :], start=True, stop=False
                )
                nc.tensor.matmul(
                    ps[:], wq_sb[:], xn_r[0:64, sl], start=False, stop=True
                )
                nc.vector.tensor_copy(q_dup[:, sl], ps[:])
            # k^T packed by half: lhsT = blockdiag(Wk, Wk)
            for t in range(4):
                sl = slice(512 * t, 512 * t + 512)
                ps = pre_ps.tile([128, 512], F32, tag="pre")
                nc.tensor.matmul(
                    ps[:], bk_sb[:], ones_sb[0:1, :], start=True, stop=False
                )
                nc.tensor.matmul(
                    ps[:], wk_sb[:], xn_r[:, sl], start=False, stop=True
                )
                nc.vector.tensor_copy(kt_sb[:, sl], ps[:])
            # v position-major, two chunks per matmul: out[pos, (half, c)]
            for u in range(16):
                sl = slice(128 * u, 128 * u + 128)
                ps = pre_ps.tile([128, 128], F32, tag="pre")
                nc.tensor.matmul(
                    ps[:], ones_sb[0:1, 0:128], bv_sb[:], start=True, stop=False
                )
                nc.tensor.matmul(
                    ps[:], xn_r[:, sl], wv_sb[:], start=False, stop=True
                )
                nc.vector.tensor_copy(v_all[:, 65 * u : 65 * u + 64], ps[:, 0:64])
                nc.vector.tensor_copy(
                    v_all[:, 65 * (u + 16) : 65 * (u + 16) + 64], ps[:, 64:128]
                )

        with tc.tile_pool(name="sc_ps", bufs=2, space="PSUM") as sc_ps, \
             tc.tile_pool(name="pj_ps", bufs=2, space="PSUM") as pj_ps, \
             tc.tile_pool(name="work", bufs=2) as work:

            batches = [list(range(t0, min(t0 + EXP_BATCH, 32)))
                       for t0 in range(0, 32, EXP_BATCH)]

            # ---- attention, one 512-query tile at a time ----
            for n in range(4):
                qsl = slice(512 * n, 512 * n + 512)
                # scores^T + exp, chunk t = 128 kv positions
                for batch in batches:
                    nb = len(batch)
                    ps = sc_ps.tile([128, 512 * EXP_BATCH], F32, tag="sc")
                    for i, t in enumerate(batch):
                        j = t // 16
                        lhsT = kt_sb[64 * j : 64 * j + 64,
                                     128 * (t % 16) : 128 * (t % 16) + 128]
                        rhs = q_dup[64 * j : 64 * j + 64, qsl]
                        nc.tensor.matmul(
                            ps[:, 512 * i : 512 * i + 512], lhsT, rhs,
                            start=True, stop=True,
                        )
                    off = 512 * batch[0]
                    nc.scalar.activation(
                        attnexp[:, off : off + 512 * nb], ps[:, 0 : 512 * nb],
                        mybir.ActivationFunctionType.Exp, scale=SCALE,
                    )
                # attn^T @ [V | 1]: accumulate over the 32 kv chunks
                pacc = pj_ps.tile([65, 512], F32, tag="pj")
                for t in range(32):
                    nc.tensor.matmul(
                        pacc[:], v_all[:, 65 * t : 65 * t + 65],
                        attnexp[:, 512 * t : 512 * t + 512],
                        start=(t == 0), stop=(t == 31),
                    )
                # normalize: proj^T = pacc[0:64] * (1/denom) broadcast
                rd = work.tile([65, 512], MDT, tag="rd")
                with nc.allow_low_precision(reason="softmax denom in f32r"):
                    nc.vector.reciprocal(rd[64:65, :], pacc[64:65, :])
                bc_ps = pj_ps.tile([64, 512], F32, tag="pj")
                nc.tensor.matmul(
                    bc_ps[:], ones_sb[64:65, 0:64], rd[64:65, :], start=True,
                    stop=True
                )
                bc_sb = work.tile([64, 512], F32, tag="bc")
                nc.vector.tensor_copy(bc_sb[:], bc_ps[:])
                projn = work.tile([64, 512], MDT, tag="projn")
                nc.vector.tensor_mul(projn[:], pacc[0:64, :], bc_sb[:])
                # out-projection + bias + residual
                fps = pj_ps.tile([64, 512], F32, tag="pj")
                nc.tensor.matmul(
                    fps[:], bo_sb[:], ones_sb[0:1, :], start=True, stop=False
                )
                nc.tensor.matmul(
                    fps[:], wo_sb[:], projn[:], start=False, stop=True
                )
                nc.vector.tensor_add(out_sb[:, qsl], fps[:], x_sb[0:64, qsl])

        nc.sync.dma_start(out_d.ap(), out_sb[:])

    nc.compile()
    return nc


def host_prep(x, gamma, beta, Wq, bq, Wk, bk, Wv, bv, Wo, bo):
    """Build the 8 per-core input dicts."""
    f32 = lambda a: np.ascontiguousarray(np.asarray(a, np.float32))
    x = f32(x)
    gamma, beta = f32(gamma), f32(beta)
    Wq, Wk, Wv, Wo = f32(Wq), f32(Wk), f32(Wv), f32(Wo)
    bq, bk, bv, bo = f32(bq), f32(bk), f32(bv), f32(bo)

    wq_dup = np.ascontiguousarray(np.concatenate([Wq, Wq], axis=1))
    z = np.zeros((64, 64), np.float32)
    wk_blk = np.ascontiguousarray(np.block([[Wk, z], [z, Wk]]))
    wv_blk = np.ascontiguousarray(np.block([[Wv, z], [z, Wv]]))
    comb = np.zeros((128, 128), np.float32)
    comb[:64, :64] = 1.0 / 64.0
    comb[64:, 64:] = 1.0 / 64.0
    mdt_np = mybir.dt.np(MDT)
    m = lambda a: np.ascontiguousarray(a).astype(mdt_np)
    shared = {
        "wq": m(wq_dup), "wk": m(wk_blk), "wv": m(wv_blk), "wo": m(Wo),
        "bq": m(np.tile(bq, 2)[None]),
        "bk": m(np.tile(bk, 2)[None]),
        "bv": m(np.tile(bv, 2)[None]),
        "bo": m(bo[None]),
        "gam": np.ascontiguousarray(np.tile(gamma, 2)[:, None]),
        "bet": np.ascontiguousarray(np.tile(beta, 2)[:, None]),
        "comb": comb,
    }
    in_maps = []
    for core in range(8):
        b, h = core // 2, core % 2
        xT = x[b].reshape(HW, C).T  # [64, 4096]
        halves = xT.reshape(C, 2, HALF)[:, [h, 1 - h], :]
        xp = np.ascontiguousarray(halves.transpose(1, 0, 2).reshape(128, HALF))
        in_maps.append({"xp": xp, **shared})
    return in_maps


def assemble(results, dtype):
    out = np.empty((B, HW, C), np.float32)
    for core in range(8):
        b, h = core // 2, core % 2
        out[b, HALF * h : HALF * h + HALF] = results[core]["out"].T
    return out.reshape(B, H, W, C).astype(dtype, copy=False)


_NC_CACHE = []


def kernel(x, gamma, beta, Wq, bq, Wk, bk, Wv, bv, Wo, bo):
    from concourse.bass_utils import run_bass_kernel_spmd

    if not _NC_CACHE:
        _NC_CACHE.append(build_nc())
    nc = _NC_CACHE[0]
    in_maps = host_prep(x, gamma, beta, Wq, bq, Wk, bk, Wv, bv, Wo, bo)
    res = run_bass_kernel_spmd(nc, in_maps, core_ids=list(range(8)))
    return assemble(res.results, np.asarray(x).dtype)


if __name__ == "__main__":
    rng = np.random.default_rng(0)
    inputs = {
        "x": rng.standard_normal((B, H, W, C)).astype(np.float32),
        "gamma": np.ones(C, np.float32), "beta": np.zeros(C, np.float32),
        "Wq": (rng.standard_normal((C, C)) / 8).astype(np.float32),
        "bq": np.zeros(C, np.float32),
        "Wk": (rng.standard_normal((C, C)) / 8).astype(np.float32),
        "bk": np.zeros(C, np.float32),
        "Wv": (rng.standard_normal((C, C)) / 8).astype(np.float32),
        "bv": np.zeros(C, np.float32),
        "Wo": (rng.standard_normal((C, C)) / 8).astype(np.float32),
        "bo": np.zeros(C, np.float32),
    }
    out = kernel(**inputs)
    print("kernel ran, out shape", out.shape, out.dtype)


# revision 9
# speedup vs baseline: 2.2980x; 1.0258x over previous
"""Trainium2 Bass kernel for nn_AttentionBlock (B=4, H=W=64, C=64, GroupNorm(8) +
full spatial self-attention), distributed over 8 NeuronCores.

Sharding: core i handles batch b=i//2 and query-half h=i%2 (2048 of the 4096
spatial positions). Each core computes the full GroupNorm and K/V for its
image (cheap) and attention only for its query half. No collectives.

Device layout: channel-on-partition ("xT") layout, with the two position
halves of an image packed onto partitions [(half, channel)] -> 128 partitions.
Scores are computed transposed (positions' on partitions) so that
- softmax reduction over positions' is a PE matmul (ones column in V), and
- the attention @ V contraction needs no transposes at all.
exp() runs on ScalarE reading PSUM and writing SBUF directly.

PE matmuls run in float32r (1 cycle/row at N>=256, ~tf32 mantissa); GroupNorm
statistics and the residual path stay full fp32.
"""

import sys

sys.path.insert(0, "/opt/trn_rl_repo")

import numpy as np

import concourse.bacc as bacc
import concourse.tile as tile
from concourse import mybir

B, H, W, C = 4, 64, 64, 64
HW = H * W  # 4096
HALF = HW // 2  # 2048
EPS = 1e-5
SCALE = C ** -0.5  # folded into exp()

F32 = mybir.dt.float32
# dtype of the PE matmul operands: float32r runs at 4x the float32 rate.
MDT = mybir.dt.bfloat16

EXP_BATCH = 3  # pos'-chunks (PSUM banks) per exp() call


def build_nc():
    nc = bacc.Bacc("TRN2", debug=False, num_devices=8)

    # ---- DRAM I/O ----
    xp_d = nc.dram_tensor("xp", [128, HALF], F32, kind="ExternalInput")
    wq_d = nc.dram_tensor("wq", [64, 128], MDT, kind="ExternalInput")
    wk_d = nc.dram_tensor("wk", [128, 128], MDT, kind="ExternalInput")
    wv_d = nc.dram_tensor("wv", [128, 128], MDT, kind="ExternalInput")
    wo_d = nc.dram_tensor("wo", [64, 64], MDT, kind="ExternalInput")
    bq_d = nc.dram_tensor("bq", [1, 128], MDT, kind="ExternalInput")
    bk_d = nc.dram_tensor("bk", [1, 128], MDT, kind="ExternalInput")
    bv_d = nc.dram_tensor("bv", [1, 128], MDT, kind="ExternalInput")
    bo_d = nc.dram_tensor("bo", [1, 64], MDT, kind="ExternalInput")
    gam_d = nc.dram_tensor("gam", [128, 1], F32, kind="ExternalInput")
    bet_d = nc.dram_tensor("bet", [128, 1], F32, kind="ExternalInput")
    comb_d = nc.dram_tensor("comb", [128, 128], F32, kind="ExternalInput")
    out_d = nc.dram_tensor("out", [64, HALF], F32, kind="ExternalOutput")

    with tile.TileContext(nc) as tc, \
         tc.tile_pool(name="singles", bufs=1) as singles, \
         tc.tile_pool(name="stats", bufs=4) as stats:

        # ---- constant loads ----
        x_sb = singles.tile([128, HALF], F32)
        nc.sync.dma_start(x_sb[:], xp_d.ap())
        wq_sb = singles.tile([64, 128], MDT)
        nc.sync.dma_start(wq_sb[:], wq_d.ap())
        wk_sb = singles.tile([128, 128], MDT)
        nc.sync.dma_start(wk_sb[:], wk_d.ap())
        wv_sb = singles.tile([128, 128], MDT)
        nc.sync.dma_start(wv_sb[:], wv_d.ap())
        wo_sb = singles.tile([64, 64], MDT)
        nc.sync.dma_start(wo_sb[:], wo_d.ap())
        bq_sb = singles.tile([1, 128], MDT)
        nc.sync.dma_start(bq_sb[:], bq_d.ap())
        bk_sb = singles.tile([1, 128], MDT)
        nc.sync.dma_start(bk_sb[:], bk_d.ap())
        bv_sb = singles.tile([1, 128], MDT)
        nc.sync.dma_start(bv_sb[:], bv_d.ap())
        bo_sb = singles.tile([1, 64], MDT)
        nc.sync.dma_start(bo_sb[:], bo_d.ap())
        gam_sb = singles.tile([128, 1], F32)
        nc.sync.dma_start(gam_sb[:], gam_d.ap())
        bet_sb = singles.tile([128, 1], F32)
        nc.sync.dma_start(bet_sb[:], bet_d.ap())
        comb_sb = singles.tile([128, 128], F32)
        nc.sync.dma_start(comb_sb[:], comb_d.ap())

        ones_f = singles.tile([128, 512], F32)
        nc.vector.memset(ones_f[:], 1.0)
        ones_sb = singles.tile([128, 512], MDT)
        nc.vector.tensor_copy(ones_sb[:], ones_f[:])

        xn_r = singles.tile([128, HALF], MDT)
        q0_sb = singles.tile([128, HALF], MDT)
        q1_sb = singles.tile([128, HALF], MDT)
        kt_sb = singles.tile([128, HALF], MDT)
        v_all = singles.tile([128, 65 * 32], MDT)
        attnexp = singles.tile([128, 512 * 32], MDT)
        out_sb = singles.tile([64, HALF], F32)

        # zero halves of the padded-query operands (once)
        zeros_f = singles.tile([64, HALF], F32)
        nc.vector.memset(zeros_f[:], 0.0)
        nc.vector.tensor_copy(q0_sb[64:128, :], zeros_f[:])
        nc.vector.tensor_copy(q1_sb[0:64, :], zeros_f[:])

        # ones column of v_all (chunk t's column 64 -> softmax denominator)
        v3 = v_all[:].rearrange("p (t e) -> p t e", e=65)
        nc.vector.tensor_copy(
            v3[:, :, 64:65], ones_f[:, 0:32].rearrange("p (a b) -> p a b", b=1)
        )

        # ---- GroupNorm (stats per partition per 512-slice, then a
        # block-diagonal averaging matmul combines across channels) ----
        smat = stats.tile([128, 8], F32)  # cols 0-3 mean, 4-7 E[x^2]
        for r in range(4):
            st6 = stats.tile([128, 6], F32, tag="st6")
            nc.vector.bn_stats(st6[:], x_sb[:, 512 * r : 512 * r + 512])
            mv = stats.tile([128, 2], F32, tag="mv")
            nc.vector.bn_aggr(mv[:], st6[:])
            nc.vector.tensor_copy(smat[:, r : r + 1], mv[:, 0:1])
            sq = stats.tile([128, 1], F32, tag="sq")
            nc.vector.tensor_mul(sq[:], mv[:, 0:1], mv[:, 0:1])
            nc.vector.tensor_add(smat[:, 4 + r : 5 + r], mv[:, 1:2], sq[:])

        with tc.tile_pool(name="pre_ps", bufs=2, space="PSUM") as pre_ps:
            cps = pre_ps.tile([128, 8], F32, tag="pre")
            nc.tensor.matmul(cps[:], comb_sb[:], smat[:], start=True, stop=True)
            gstat = stats.tile([128, 8], F32)  # 0-3 mean_g, 4-7 E2_g
            nc.vector.tensor_copy(gstat[:], cps[:])

            var_g = stats.tile([128, 4], F32)
            nc.vector.tensor_mul(var_g[:], gstat[:, 0:4], gstat[:, 0:4])
            nc.vector.tensor_tensor(
                var_g[:], gstat[:, 4:8], var_g[:], op=mybir.AluOpType.subtract
            )
            # rstd = exp(-0.5 * ln(var + eps)); Ln/Exp share one ACT table set
            eps_sb = stats.tile([128, 1], F32)
            nc.vector.memset(eps_sb[:], EPS)
            rstd = stats.tile([128, 4], F32)
            nc.scalar.activation(
                rstd[:], var_g[:], mybir.ActivationFunctionType.Ln, bias=eps_sb[:]
            )
            nc.scalar.activation(
                rstd[:], rstd[:], mybir.ActivationFunctionType.Exp, scale=-0.5
            )
            gsc = stats.tile([128, 4], F32)
            nc.vector.tensor_scalar_mul(gsc[:], rstd[:], gam_sb[:])
            gbias = stats.tile([128, 4], F32)
            nc.vector.tensor_mul(gbias[:], gstat[:, 0:4], gsc[:])
            nc.vector.tensor_scalar(
                out=gbias[:], in0=gbias[:], scalar1=-1.0, scalar2=bet_sb[:],
                op0=mybir.AluOpType.mult, op1=mybir.AluOpType.add,
            )
            # xn = x * gsc + gbias: fp32 in place (residual path) and rounded
            # f32r copy (matmul path)
            for r in range(4):
                nc.vector.tensor_scalar(
                    out=x_sb[:, 512 * r : 512 * r + 512],
                    in0=x_sb[:, 512 * r : 512 * r + 512],
                    scalar1=gsc[:, r : r + 1], scalar2=gbias[:, r : r + 1],
                    op0=mybir.AluOpType.mult, op1=mybir.AluOpType.add,
                )
            nc.vector.tensor_copy(xn_r[:], x_sb[:])

            # ---- Q/K/V projections ----
            # q^T duplicated on both partition halves: lhsT = [Wq | Wq]
            for t in range(4):
                sl = slice(512 * t, 512 * t + 512)
                ps = pre_ps.tile([128, 512], F32, tag="pre")
                nc.tensor.matmul(
                    ps[:], bq_sb[:], ones_sb[0:1, :], start=True, stop=False
                )
                nc.tensor.matmul(
                    ps[:], wq_sb[:], xn_r[0:64, sl], start=False, stop=True
                )
                nc.vector.tensor_copy(q0_sb[0:64, sl], ps[0:64, :])
                nc.vector.tensor_copy(q1_sb[64:128, sl], ps[64:128, :])
            # k^T packed by half: lhsT = blockdiag(Wk, Wk)
            for t in range(4):
                sl = slice(512 * t, 512 * t + 512)
                ps = pre_ps.tile([128, 512], F32, tag="pre")
                nc.tensor.matmul(
                    ps[:], bk_sb[:], ones_sb[0:1, :], start=True, stop=False
                )
                nc.tensor.matmul(
                    ps[:], wk_sb[:], xn_r[:, sl], start=False, stop=True
                )
                nc.vector.tensor_copy(kt_sb[:, sl], ps[:])
            # v position-major, two chunks per matmul: out[pos, (half, c)]
            for u in range(16):
                sl = slice(128 * u, 128 * u + 128)
                ps = pre_ps.tile([128, 128], F32, tag="pre")
                nc.tensor.matmul(
                    ps[:], ones_sb[0:1, 0:128], bv_sb[:], start=True, stop=False
                )
                nc.tensor.matmul(
                    ps[:], xn_r[:, sl], wv_sb[:], start=False, stop=True
                )
                nc.vector.tensor_copy(v_all[:, 65 * u : 65 * u + 64], ps[:, 0:64])
                nc.vector.tensor_copy(
                    v_all[:, 65 * (u + 16) : 65 * (u + 16) + 64], ps[:, 64:128]
                )

        with tc.tile_pool(name="sc_ps", bufs=2, space="PSUM") as sc_ps, \
             tc.tile_pool(name="pj_ps", bufs=2, space="PSUM") as pj_ps, \
             tc.tile_pool(name="work", bufs=2) as work:

            batches = [list(range(t0, min(t0 + EXP_BATCH, 32)))
                       for t0 in range(0, 32, EXP_BATCH)]

            # ---- attention, one 512-query tile at a time ----
            for n in range(4):
                qsl = slice(512 * n, 512 * n + 512)
                # scores^T + exp, chunk t = 128 kv positions
                for batch in batches:
                    nb = len(batch)
                    ps = sc_ps.tile([128, 512 * EXP_BATCH], F32, tag="sc")
                    for i, t in enumerate(batch):
                        lhsT = kt_sb[:, 128 * (t % 16) : 128 * (t % 16) + 128]
                        rhs = (q0_sb if t < 16 else q1_sb)[:, qsl]
                        nc.tensor.matmul(
                            ps[:, 512 * i : 512 * i + 512], lhsT, rhs,
                            start=True, stop=True,
                        )
                    off = 512 * batch[0]
                    nc.scalar.activation(
                        attnexp[:, off : off + 512 * nb], ps[:, 0 : 512 * nb],
                        mybir.ActivationFunctionType.Exp, scale=SCALE,
                    )
                # attn^T @ [V | 1]: accumulate over the 32 kv chunks
                pacc = pj_ps.tile([65, 512], F32, tag="pj")
                for t in range(32):
                    nc.tensor.matmul(
                        pacc[:], v_all[:, 65 * t : 65 * t + 65],
                        attnexp[:, 512 * t : 512 * t + 512],
                        start=(t == 0), stop=(t == 31),
                    )
                # normalize: proj^T = pacc[0:64] * (1/denom) broadcast
                rd = work.tile([65, 512], MDT, tag="rd")
                with nc.allow_low_precision(reason="softmax denom in f32r"):
                    nc.vector.reciprocal(rd[64:65, :], pacc[64:65, :])
                bc_ps = pj_ps.tile([64, 512], F32, tag="pj")
                nc.tensor.matmul(
                    bc_ps[:], ones_sb[64:65, 0:64], rd[64:65, :], start=True,
                    stop=True
                )
                bc_sb = work.tile([64, 512], F32, tag="bc")
                nc.vector.tensor_copy(bc_sb[:], bc_ps[:])
                projn = work.tile([64, 512], MDT, tag="projn")
                nc.vector.tensor_mul(projn[:], pacc[0:64, :], bc_sb[:])
                # out-projection + bias + residual
                fps = pj_ps.tile([64, 512], F32, tag="pj")
                nc.tensor.matmul(
                    fps[:], bo_sb[:], ones_sb[0:1, :], start=True, stop=False
                )
                nc.tensor.matmul(
                    fps[:], wo_sb[:], projn[:], start=False, stop=True
                )
                nc.vector.tensor_add(out_sb[:, qsl], fps[:], x_sb[0:64, qsl])

        nc.sync.dma_start(out_d.ap(), out_sb[:])

    nc.compile()
    return nc


def host_prep(x, gamma, beta, Wq, bq, Wk, bk, Wv, bv, Wo, bo):
    """Build the 8 per-core input dicts."""
    f32 = lambda a: np.ascontiguousarray(np.asarray(a, np.float32))
    x = f32(x)
    gamma, beta = f32(gamma), f32(beta)
    Wq, Wk, Wv, Wo = f32(Wq), f32(Wk), f32(Wv), f32(Wo)
    bq, bk, bv, bo = f32(bq), f32(bk), f32(bv), f32(bo)

    wq_dup = np.ascontiguousarray(np.concatenate([Wq, Wq], axis=1))
    z = np.zeros((64, 64), np.float32)
    wk_blk = np.ascontiguousarray(np.block([[Wk, z], [z, Wk]]))
    wv_blk = np.ascontiguousarray(np.block([[Wv, z], [z, Wv]]))
    comb = np.zeros((128, 128), np.float32)
    comb[:64, :64] = 1.0 / 64.0
    comb[64:, 64:] = 1.0 / 64.0
    mdt_np = mybir.dt.np(MDT)
    m = lambda a: np.ascontiguousarray(a).astype(mdt_np)
    shared = {
        "wq": m(wq_dup), "wk": m(wk_blk), "wv": m(wv_blk), "wo": m(Wo),
        "bq": m(np.tile(bq, 2)[None]),
        "bk": m(np.tile(bk, 2)[None]),
        "bv": m(np.tile(bv, 2)[None]),
        "bo": m(bo[None]),
        "gam": np.ascontiguousarray(np.tile(gamma, 2)[:, None]),
        "bet": np.ascontiguousarray(np.tile(beta, 2)[:, None]),
        "comb": comb,
    }
    in_maps = []
    for core in range(8):
        b, h = core // 2, core % 2
        xT = x[b].reshape(HW, C).T  # [64, 4096]
        halves = xT.reshape(C, 2, HALF)[:, [h, 1 - h], :]
        xp = np.ascontiguousarray(halves.transpose(1, 0, 2).reshape(128, HALF))
        in_maps.append({"xp": xp, **shared})
    return in_maps


def assemble(results, dtype):
    out = np.empty((B, HW, C), np.float32)
    for core in range(8):
        b, h = core // 2, core % 2
        out[b, HALF * h : HALF * h + HALF] = results[core]["out"].T
    return out.reshape(B, H, W, C).astype(dtype, copy=False)


_NC_CACHE = []


def kernel(x, gamma, beta, Wq, bq, Wk, bk, Wv, bv, Wo, bo):
    from concourse.bass_utils import run_bass_kernel_spmd

    if not _NC_CACHE:
        _NC_CACHE.append(build_nc())
    nc = _NC_CACHE[0]
    in_maps = host_prep(x, gamma, beta, Wq, bq, Wk, bk, Wv, bv, Wo, bo)
    res = run_bass_kernel_spmd(nc, in_maps, core_ids=list(range(8)))
    return assemble(res.results, np.asarray(x).dtype)


if __name__ == "__main__":
    rng = np.random.default_rng(0)
    inputs = {
        "x": rng.standard_normal((B, H, W, C)).astype(np.float32),
        "gamma": np.ones(C, np.float32), "beta": np.zeros(C, np.float32),
        "Wq": (rng.standard_normal((C, C)) / 8).astype(np.float32),
        "bq": np.zeros(C, np.float32),
        "Wk": (rng.standard_normal((C, C)) / 8).astype(np.float32),
        "bk": np.zeros(C, np.float32),
        "Wv": (rng.standard_normal((C, C)) / 8).astype(np.float32),
        "bv": np.zeros(C, np.float32),
        "Wo": (rng.standard_normal((C, C)) / 8).astype(np.float32),
        "bo": np.zeros(C, np.float32),
    }
    out = kernel(**inputs)
    print("kernel ran, out shape", out.shape, out.dtype)


# revision 11
# speedup vs baseline: 2.3176x; 1.0086x over previous
"""Trainium2 Bass kernel for nn_AttentionBlock (B=4, H=W=64, C=64, GroupNorm(8) +
full spatial self-attention), distributed over 8 NeuronCores.

Sharding: core i handles batch b=i//2 and query-half h=i%2 (2048 of the 4096
spatial positions). Each core computes the full GroupNorm and K/V for its
image (cheap) and attention only for its query half. No collectives.

Device layout: channel-on-partition ("xT") layout, with the two position
halves of an image packed onto partitions [(half, channel)] -> 128 partitions.
Scores are computed transposed (positions' on partitions) so that
- softmax reduction over positions' is a PE matmul (ones column in V), and
- the attention @ V contraction needs no transposes at all.
exp() runs on ScalarE reading PSUM and writing SBUF directly.

PE matmuls run in float32r (1 cycle/row at N>=256, ~tf32 mantissa); GroupNorm
statistics and the residual path stay full fp32.
"""

import sys

sys.path.insert(0, "/opt/trn_rl_repo")

import numpy as np

import concourse.bacc as bacc
import concourse.tile as tile
from concourse import mybir

B, H, W, C = 4, 64, 64, 64
HW = H * W  # 4096
HALF = HW // 2  # 2048
EPS = 1e-5
SCALE = C ** -0.5  # folded into exp()

F32 = mybir.dt.float32
# dtype of the PE matmul operands: float32r runs at 4x the float32 rate.
MDT = mybir.dt.bfloat16

EXP_BATCH = 3  # pos'-chunks (PSUM banks) per exp() call


def build_nc():
    nc = bacc.Bacc("TRN2", debug=False, num_devices=8)

    # ---- DRAM I/O ----
    xp_d = nc.dram_tensor("xp", [128, HALF], F32, kind="ExternalInput")
    wq_d = nc.dram_tensor("wq", [64, 128], MDT, kind="ExternalInput")
    wk_d = nc.dram_tensor("wk", [128, 128], MDT, kind="ExternalInput")
    wv_d = nc.dram_tensor("wv", [128, 128], MDT, kind="ExternalInput")
    wo_d = nc.dram_tensor("wo", [64, 64], MDT, kind="ExternalInput")
    bq_d = nc.dram_tensor("bq", [1, 128], MDT, kind="ExternalInput")
    bk_d = nc.dram_tensor("bk", [1, 128], MDT, kind="ExternalInput")
    bv_d = nc.dram_tensor("bv", [1, 128], MDT, kind="ExternalInput")
    bo_d = nc.dram_tensor("bo", [1, 64], MDT, kind="ExternalInput")
    gam_d = nc.dram_tensor("gam", [128, 1], F32, kind="ExternalInput")
    bet_d = nc.dram_tensor("bet", [128, 1], F32, kind="ExternalInput")
    comb_d = nc.dram_tensor("comb", [128, 128], F32, kind="ExternalInput")
    out_d = nc.dram_tensor("out", [64, HALF], F32, kind="ExternalOutput")

    with tile.TileContext(nc) as tc, \
         tc.tile_pool(name="singles", bufs=1) as singles, \
         tc.tile_pool(name="stats", bufs=4) as stats:

        # ---- constant loads ----
        x_sb = singles.tile([128, HALF], F32)
        nc.sync.dma_start(x_sb[:], xp_d.ap())
        wq_sb = singles.tile([64, 128], MDT)
        nc.sync.dma_start(wq_sb[:], wq_d.ap())
        wk_sb = singles.tile([128, 128], MDT)
        nc.sync.dma_start(wk_sb[:], wk_d.ap())
        wv_sb = singles.tile([128, 128], MDT)
        nc.sync.dma_start(wv_sb[:], wv_d.ap())
        wo_sb = singles.tile([64, 64], MDT)
        nc.sync.dma_start(wo_sb[:], wo_d.ap())
        bq_sb = singles.tile([1, 128], MDT)
        nc.sync.dma_start(bq_sb[:], bq_d.ap())
        bk_sb = singles.tile([1, 128], MDT)
        nc.sync.dma_start(bk_sb[:], bk_d.ap())
        bv_sb = singles.tile([1, 128], MDT)
        nc.sync.dma_start(bv_sb[:], bv_d.ap())
        bo_sb = singles.tile([1, 64], MDT)
        nc.sync.dma_start(bo_sb[:], bo_d.ap())
        gam_sb = singles.tile([128, 1], F32)
        nc.sync.dma_start(gam_sb[:], gam_d.ap())
        bet_sb = singles.tile([128, 1], F32)
        nc.sync.dma_start(bet_sb[:], bet_d.ap())
        comb_sb = singles.tile([128, 128], F32)
        nc.sync.dma_start(comb_sb[:], comb_d.ap())

        ones_f = singles.tile([128, 512], F32)
        nc.vector.memset(ones_f[:], 1.0)
        ones_sb = singles.tile([128, 512], MDT)
        nc.vector.tensor_copy(ones_sb[:], ones_f[:])

        xn_r = singles.tile([128, HALF], MDT)
        q0_sb = singles.tile([128, HALF], MDT)
        q1_sb = singles.tile([128, HALF], MDT)
        kt_sb = singles.tile([128, HALF], MDT)
        v_all = singles.tile([128, 65 * 32], MDT)
        attnexp_a = singles.tile([128, 512 * 32], MDT)
        attnexp_b = singles.tile([128, 512 * 32], MDT)
        out_sb = singles.tile([64, HALF], F32)

        # zero halves of the padded-query operands (once)
        zeros_f = singles.tile([64, HALF], F32)
        nc.vector.memset(zeros_f[:], 0.0)
        nc.vector.tensor_copy(q0_sb[64:128, :], zeros_f[:])
        nc.vector.tensor_copy(q1_sb[0:64, :], zeros_f[:])

        # ones column of v_all (chunk t's column 64 -> softmax denominator)
        v3 = v_all[:].rearrange("p (t e) -> p t e", e=65)
        nc.vector.tensor_copy(
            v3[:, :, 64:65], ones_f[:, 0:32].rearrange("p (a b) -> p a b", b=1)
        )

        # ---- GroupNorm (stats per partition per 512-slice, then a
        # block-diagonal averaging matmul combines across channels) ----
        smat = stats.tile([128, 8], F32)  # cols 0-3 mean, 4-7 E[x^2]
        for r in range(4):
            st6 = stats.tile([128, 6], F32, tag="st6")
            nc.vector.bn_stats(st6[:], x_sb[:, 512 * r : 512 * r + 512])
            mv = stats.tile([128, 2], F32, tag="mv")
            nc.vector.bn_aggr(mv[:], st6[:])
            nc.vector.tensor_copy(smat[:, r : r + 1], mv[:, 0:1])
            sq = stats.tile([128, 1], F32, tag="sq")
            nc.vector.tensor_mul(sq[:], mv[:, 0:1], mv[:, 0:1])
            nc.vector.tensor_add(smat[:, 4 + r : 5 + r], mv[:, 1:2], sq[:])

        with tc.tile_pool(name="pre_ps", bufs=2, space="PSUM") as pre_ps:
            cps = pre_ps.tile([128, 8], F32, tag="pre")
            nc.tensor.matmul(cps[:], comb_sb[:], smat[:], start=True, stop=True)
            gstat = stats.tile([128, 8], F32)  # 0-3 mean_g, 4-7 E2_g
            nc.vector.tensor_copy(gstat[:], cps[:])

            var_g = stats.tile([128, 4], F32)
            nc.vector.tensor_mul(var_g[:], gstat[:, 0:4], gstat[:, 0:4])
            nc.vector.tensor_tensor(
                var_g[:], gstat[:, 4:8], var_g[:], op=mybir.AluOpType.subtract
            )
            # rstd = exp(-0.5 * ln(var + eps)); Ln/Exp share one ACT table set
            eps_sb = stats.tile([128, 1], F32)
            nc.vector.memset(eps_sb[:], EPS)
            rstd = stats.tile([128, 4], F32)
            nc.scalar.activation(
                rstd[:], var_g[:], mybir.ActivationFunctionType.Ln, bias=eps_sb[:]
            )
            nc.scalar.activation(
                rstd[:], rstd[:], mybir.ActivationFunctionType.Exp, scale=-0.5
            )
            gsc = stats.tile([128, 4], F32)
            nc.vector.tensor_scalar_mul(gsc[:], rstd[:], gam_sb[:])
            gbias = stats.tile([128, 4], F32)
            nc.vector.tensor_mul(gbias[:], gstat[:, 0:4], gsc[:])
            nc.vector.tensor_scalar(
                out=gbias[:], in0=gbias[:], scalar1=-1.0, scalar2=bet_sb[:],
                op0=mybir.AluOpType.mult, op1=mybir.AluOpType.add,
            )
            # xn = x * gsc + gbias: fp32 in place (residual path) and rounded
            # f32r copy (matmul path)
            for r in range(4):
                nc.vector.tensor_scalar(
                    out=x_sb[:, 512 * r : 512 * r + 512],
                    in0=x_sb[:, 512 * r : 512 * r + 512],
                    scalar1=gsc[:, r : r + 1], scalar2=gbias[:, r : r + 1],
                    op0=mybir.AluOpType.mult, op1=mybir.AluOpType.add,
                )
            nc.vector.tensor_copy(xn_r[:], x_sb[:])

            # ---- Q/K/V projections ----
            # q^T duplicated on both partition halves: lhsT = [Wq | Wq]
            for t in range(4):
                sl = slice(512 * t, 512 * t + 512)
                ps = pre_ps.tile([128, 512], F32, tag="pre")
                nc.tensor.matmul(
                    ps[:], bq_sb[:], ones_sb[0:1, :], start=True, stop=False
                )
                nc.tensor.matmul(
                    ps[:], wq_sb[:], xn_r[0:64, sl], start=False, stop=True
                )
                nc.vector.tensor_copy(q0_sb[0:64, sl], ps[0:64, :])
                nc.vector.tensor_copy(q1_sb[64:128, sl], ps[64:128, :])
            # k^T packed by half: lhsT = blockdiag(Wk, Wk)
            for t in range(4):
                sl = slice(512 * t, 512 * t + 512)
                ps = pre_ps.tile([128, 512], F32, tag="pre")
                nc.tensor.matmul(
                    ps[:], bk_sb[:], ones_sb[0:1, :], start=True, stop=False
                )
                nc.tensor.matmul(
                    ps[:], wk_sb[:], xn_r[:, sl], start=False, stop=True
                )
                nc.vector.tensor_copy(kt_sb[:, sl], ps[:])
            # v position-major, two chunks per matmul: out[pos, (half, c)]
            for u in range(16):
                sl = slice(128 * u, 128 * u + 128)
                ps = pre_ps.tile([128, 128], F32, tag="pre")
                nc.tensor.matmul(
                    ps[:], ones_sb[0:1, 0:128], bv_sb[:], start=True, stop=False
                )
                nc.tensor.matmul(
                    ps[:], xn_r[:, sl], wv_sb[:], start=False, stop=True
                )
                nc.vector.tensor_copy(v_all[:, 65 * u : 65 * u + 64], ps[:, 0:64])
                nc.vector.tensor_copy(
                    v_all[:, 65 * (u + 16) : 65 * (u + 16) + 64], ps[:, 64:128]
                )

        with tc.tile_pool(name="sc_ps", bufs=2, space="PSUM") as sc_ps, \
             tc.tile_pool(name="pj_ps", bufs=2, space="PSUM") as pj_ps, \
             tc.tile_pool(name="work", bufs=2) as work:

            batches = [list(range(t0, min(t0 + EXP_BATCH, 32)))
                       for t0 in range(0, 32, EXP_BATCH)]
            attn_bufs = [attnexp_a, attnexp_b]
            paccs = [None, None, None, None]

            def emit_scores_batch(n, batch):
                attnexp = attn_bufs[n % 2]
                qsl = slice(512 * n, 512 * n + 512)
                nb = len(batch)
                ps = sc_ps.tile([128, 512 * EXP_BATCH], F32, tag="sc")
                for i, t in enumerate(batch):
                    lhsT = kt_sb[:, 128 * (t % 16) : 128 * (t % 16) + 128]
                    rhs = (q0_sb if t < 16 else q1_sb)[:, qsl]
                    nc.tensor.matmul(
                        ps[:, 512 * i : 512 * i + 512], lhsT, rhs,
                        start=True, stop=True,
                    )
                off = 512 * batch[0]
                nc.scalar.activation(
                    attnexp[:, off : off + 512 * nb], ps[:, 0 : 512 * nb],
                    mybir.ActivationFunctionType.Exp, scale=SCALE,
                )

            def emit_proj_batch(n, batch):
                attnexp = attn_bufs[n % 2]
                if paccs[n] is None:
                    paccs[n] = pj_ps.tile([65, 512], F32, tag="pj", name=f"pacc{n}")
                pacc = paccs[n]
                for t in batch:
                    nc.tensor.matmul(
                        pacc[:], v_all[:, 65 * t : 65 * t + 65],
                        attnexp[:, 512 * t : 512 * t + 512],
                        start=(t == 0), stop=(t == 31),
                    )

            def emit_finish(n):
                # normalize: proj^T = pacc[0:64] * (1/denom) broadcast,
                # then out-projection + bias + residual
                pacc = paccs[n]
                qsl = slice(512 * n, 512 * n + 512)
                rd = work.tile([65, 512], MDT, tag="rd")
                with nc.allow_low_precision(reason="softmax denom in f32r"):
                    nc.vector.reciprocal(rd[64:65, :], pacc[64:65, :])
                bc_ps = pj_ps.tile([64, 512], F32, tag="pj")
                nc.tensor.matmul(
                    bc_ps[:], ones_sb[64:65, 0:64], rd[64:65, :], start=True,
                    stop=True
                )
                bc_sb = work.tile([64, 512], F32, tag="bc")
                nc.vector.tensor_copy(bc_sb[:], bc_ps[:])
                projn = work.tile([64, 512], MDT, tag="projn")
                nc.vector.tensor_mul(projn[:], pacc[0:64, :], bc_sb[:])
                fps = pj_ps.tile([64, 512], F32, tag="pj")
                nc.tensor.matmul(
                    fps[:], bo_sb[:], ones_sb[0:1, :], start=True, stop=False
                )
                nc.tensor.matmul(
                    fps[:], wo_sb[:], projn[:], start=False, stop=True
                )
                nc.vector.tensor_add(out_sb[:, qsl], fps[:], x_sb[0:64, qsl])

            # ---- software-pipelined attention: scores/exp of tile n
            # interleaved with attn@V of tile n-1 ----
            for n in range(4):
                for bi, batch in enumerate(batches):
                    emit_scores_batch(n, batch)
                    if n > 0:
                        emit_proj_batch(n - 1, batch)
                if n > 0:
                    emit_finish(n - 1)
            for batch in batches:
                emit_proj_batch(3, batch)
            emit_finish(3)

        nc.sync.dma_start(out_d.ap(), out_sb[:])

    nc.compile()
    return nc


def host_prep(x, gamma, beta, Wq, bq, Wk, bk, Wv, bv, Wo, bo):
    """Build the 8 per-core input dicts."""
    f32 = lambda a: np.ascontiguousarray(np.asarray(a, np.float32))
    x = f32(x)
    gamma, beta = f32(gamma), f32(beta)
    Wq, Wk, Wv, Wo = f32(Wq), f32(Wk), f32(Wv), f32(Wo)
    bq, bk, bv, bo = f32(bq), f32(bk), f32(bv), f32(bo)

    wq_dup = np.ascontiguousarray(np.concatenate([Wq, Wq], axis=1))
    z = np.zeros((64, 64), np.float32)
    wk_blk = np.ascontiguousarray(np.block([[Wk, z], [z, Wk]]))
    wv_blk = np.ascontiguousarray(np.block([[Wv, z], [z, Wv]]))
    comb = np.zeros((128, 128), np.float32)
    comb[:64, :64] = 1.0 / 64.0
    comb[64:, 64:] = 1.0 / 64.0
    mdt_np = mybir.dt.np(MDT)
    m = lambda a: np.ascontiguousarray(a).astype(mdt_np)
    shared = {
        "wq": m(wq_dup), "wk": m(wk_blk), "wv": m(wv_blk), "wo": m(Wo),
        "bq": m(np.tile(bq, 2)[None]),
        "bk": m(np.tile(bk, 2)[None]),
        "bv": m(np.tile(bv, 2)[None]),
        "bo": m(bo[None]),
        "gam": np.ascontiguousarray(np.tile(gamma, 2)[:, None]),
        "bet": np.ascontiguousarray(np.tile(beta, 2)[:, None]),
        "comb": comb,
    }
    in_maps = []
    for core in range(8):
        b, h = core // 2, core % 2
        xT = x[b].reshape(HW, C).T  # [64, 4096]
        halves = xT.reshape(C, 2, HALF)[:, [h, 1 - h], :]
        xp = np.ascontiguousarray(halves.transpose(1, 0, 2).reshape(128, HALF))
        in_maps.append({"xp": xp, **shared})
    return in_maps


def assemble(results, dtype):
    out = np.empty((B, HW, C), np.float32)
    for core in range(8):
        b, h = core // 2, core % 2
        out[b, HALF * h : HALF * h + HALF] = results[core]["out"].T
    return out.reshape(B, H, W, C).astype(dtype, copy=False)


_NC_CACHE = []


def kernel(x, gamma, beta, Wq, bq, Wk, bk, Wv, bv, Wo, bo):
    from concourse.bass_utils import run_bass_kernel_spmd

    if not _NC_CACHE:
        _NC_CACHE.append(build_nc())
    nc = _NC_CACHE[0]
    in_maps = host_prep(x, gamma, beta, Wq, bq, Wk, bk, Wv, bv, Wo, bo)
    res = run_bass_kernel_spmd(nc, in_maps, core_ids=list(range(8)))
    return assemble(res.results, np.asarray(x).dtype)


if __name__ == "__main__":
    rng = np.random.default_rng(0)
    inputs = {
        "x": rng.standard_normal((B, H, W, C)).astype(np.float32),
        "gamma": np.ones(C, np.float32), "beta": np.zeros(C, np.float32),
        "Wq": (rng.standard_normal((C, C)) / 8).astype(np.float32),
        "bq": np.zeros(C, np.float32),
        "Wk": (rng.standard_normal((C, C)) / 8).astype(np.float32),
        "bk": np.zeros(C, np.float32),
        "Wv": (rng.standard_normal((C, C)) / 8).astype(np.float32),
        "bv": np.zeros(C, np.float32),
        "Wo": (rng.standard_normal((C, C)) / 8).astype(np.float32),
        "bo": np.zeros(C, np.float32),
    }
    out = kernel(**inputs)
    print("kernel ran, out shape", out.shape, out.dtype)


# revision 12
# speedup vs baseline: 2.7537x; 1.1882x over previous
"""Trainium2 Bass kernel for nn_AttentionBlock (B=4, H=W=64, C=64, GroupNorm(8) +
full spatial self-attention), distributed over 8 NeuronCores.

Sharding: core i handles batch b=i//2 and query-half h=i%2 (2048 of the 4096
spatial positions). Each core computes the full GroupNorm and K/V for its
image (cheap) and attention only for its query half. No collectives.

Device layout: channel-on-partition ("xT") layout, with the two position
halves of an image packed onto partitions [(half, channel)] -> 128 partitions.
Scores are computed transposed (positions' on partitions) so that
- softmax reduction over positions' is a PE matmul (ones column in V), and
- the attention @ V contraction needs no transposes at all.
exp() runs on ScalarE reading PSUM and writing SBUF directly.

PE matmuls run in float32r (1 cycle/row at N>=256, ~tf32 mantissa); GroupNorm
statistics and the residual path stay full fp32.
"""

import sys

sys.path.insert(0, "/opt/trn_rl_repo")

import numpy as np

import concourse.bacc as bacc
import concourse.tile as tile
from concourse import mybir

B, H, W, C = 4, 64, 64, 64
HW = H * W  # 4096
HALF = HW // 2  # 2048
EPS = 1e-5
SCALE = C ** -0.5  # folded into exp()

F32 = mybir.dt.float32
# dtype of the PE matmul operands: float32r runs at 4x the float32 rate.
MDT = mybir.dt.bfloat16

EXP_BATCH = 3  # pos'-chunks (PSUM banks) per exp() call


def build_nc():
    nc = bacc.Bacc("TRN2", debug=False, num_devices=8)

    # ---- DRAM I/O ----
    xp_d = nc.dram_tensor("xp", [128, HALF], F32, kind="ExternalInput")
    wq_d = nc.dram_tensor("wq", [64, 128], MDT, kind="ExternalInput")
    wk_d = nc.dram_tensor("wk", [128, 128], MDT, kind="ExternalInput")
    wv_d = nc.dram_tensor("wv", [128, 128], MDT, kind="ExternalInput")
    wo_d = nc.dram_tensor("wo", [64, 64], MDT, kind="ExternalInput")
    bq_d = nc.dram_tensor("bq", [1, 128], MDT, kind="ExternalInput")
    bk_d = nc.dram_tensor("bk", [1, 128], MDT, kind="ExternalInput")
    bv_d = nc.dram_tensor("bv", [1, 128], MDT, kind="ExternalInput")
    bo_d = nc.dram_tensor("bo", [1, 64], MDT, kind="ExternalInput")
    gam_d = nc.dram_tensor("gam", [128, 1], F32, kind="ExternalInput")
    bet_d = nc.dram_tensor("bet", [128, 1], F32, kind="ExternalInput")
    comb_d = nc.dram_tensor("comb", [128, 128], F32, kind="ExternalInput")
    out_d = nc.dram_tensor("out", [64, HALF], F32, kind="ExternalOutput")

    with tile.TileContext(nc) as tc, \
         tc.tile_pool(name="singles", bufs=1) as singles, \
         tc.tile_pool(name="stats", bufs=4) as stats:

        # ---- constant loads ----
        x_sb = singles.tile([128, HALF], F32)
        for r in range(4):
            nc.sync.dma_start(
                x_sb[:, 512 * r : 512 * r + 512],
                xp_d.ap()[:, 512 * r : 512 * r + 512],
            )
        wq_sb = singles.tile([64, 128], MDT)
        nc.sync.dma_start(wq_sb[:], wq_d.ap())
        wk_sb = singles.tile([128, 128], MDT)
        nc.sync.dma_start(wk_sb[:], wk_d.ap())
        wv_sb = singles.tile([128, 128], MDT)
        nc.sync.dma_start(wv_sb[:], wv_d.ap())
        wo_sb = singles.tile([64, 64], MDT)
        nc.sync.dma_start(wo_sb[:], wo_d.ap())
        bq_sb = singles.tile([1, 128], MDT)
        nc.sync.dma_start(bq_sb[:], bq_d.ap())
        bk_sb = singles.tile([1, 128], MDT)
        nc.sync.dma_start(bk_sb[:], bk_d.ap())
        bv_sb = singles.tile([1, 128], MDT)
        nc.sync.dma_start(bv_sb[:], bv_d.ap())
        bo_sb = singles.tile([1, 64], MDT)
        nc.sync.dma_start(bo_sb[:], bo_d.ap())
        gam_sb = singles.tile([128, 1], F32)
        nc.sync.dma_start(gam_sb[:], gam_d.ap())
        bet_sb = singles.tile([128, 1], F32)
        nc.sync.dma_start(bet_sb[:], bet_d.ap())
        comb_sb = singles.tile([128, 128], F32)
        nc.sync.dma_start(comb_sb[:], comb_d.ap())

        ones_f = singles.tile([128, 512], F32)
        nc.vector.memset(ones_f[:], 1.0)
        ones_sb = singles.tile([128, 512], MDT)
        nc.vector.tensor_copy(ones_sb[:], ones_f[:])

        xn_r = singles.tile([128, HALF], MDT)
        q0_sb = singles.tile([128, HALF], MDT)
        q1_sb = singles.tile([128, HALF], MDT)
        kt_sb = singles.tile([128, HALF], MDT)
        v_all = singles.tile([128, 65 * 32], MDT)
        attnexp_a = singles.tile([128, 512 * 32], MDT)
        attnexp_b = singles.tile([128, 512 * 32], MDT)
        out_sb = singles.tile([64, HALF], F32)

        # zero halves of the padded-query operands (once)
        zeros_f = singles.tile([64, HALF], F32)
        nc.vector.memset(zeros_f[:], 0.0)
        nc.vector.tensor_copy(q0_sb[64:128, :], zeros_f[:])
        nc.vector.tensor_copy(q1_sb[0:64, :], zeros_f[:])

        # ones column of v_all (chunk t's column 64 -> softmax denominator)
        v3 = v_all[:].rearrange("p (t e) -> p t e", e=65)
        nc.vector.tensor_copy(
            v3[:, :, 64:65], ones_f[:, 0:32].rearrange("p (a b) -> p a b", b=1)
        )

        # ---- GroupNorm (stats per partition per 512-slice, then a
        # block-diagonal averaging matmul combines across channels) ----
        smat = stats.tile([128, 8], F32)  # cols 0-3 mean, 4-7 E[x^2]
        for r in range(4):
            st6 = stats.tile([128, 6], F32, tag="st6")
            nc.vector.bn_stats(st6[:], x_sb[:, 512 * r : 512 * r + 512])
            mv = stats.tile([128, 2], F32, tag="mv")
            nc.vector.bn_aggr(mv[:], st6[:])
            nc.vector.tensor_copy(smat[:, r : r + 1], mv[:, 0:1])
            sq = stats.tile([128, 1], F32, tag="sq")
            nc.vector.tensor_mul(sq[:], mv[:, 0:1], mv[:, 0:1])
            nc.vector.tensor_add(smat[:, 4 + r : 5 + r], mv[:, 1:2], sq[:])

        with tc.tile_pool(name="pre_ps", bufs=2, space="PSUM") as pre_ps:
            cps = pre_ps.tile([128, 8], F32, tag="pre")
            nc.tensor.matmul(cps[:], comb_sb[:], smat[:], start=True, stop=True)
            gstat = stats.tile([128, 8], F32)  # 0-3 mean_g, 4-7 E2_g
            nc.vector.tensor_copy(gstat[:], cps[:])

            var_g = stats.tile([128, 4], F32)
            nc.vector.tensor_mul(var_g[:], gstat[:, 0:4], gstat[:, 0:4])
            nc.vector.tensor_tensor(
                var_g[:], gstat[:, 4:8], var_g[:], op=mybir.AluOpType.subtract
            )
            # rstd = exp(-0.5 * ln(var + eps)); Ln/Exp share one ACT table set
            eps_sb = stats.tile([128, 1], F32)
            nc.vector.memset(eps_sb[:], EPS)
            rstd = stats.tile([128, 4], F32)
            nc.scalar.activation(
                rstd[:], var_g[:], mybir.ActivationFunctionType.Ln, bias=eps_sb[:]
            )
            nc.scalar.activation(
                rstd[:], rstd[:], mybir.ActivationFunctionType.Exp, scale=-0.5
            )
            gsc = stats.tile([128, 4], F32)
            nc.vector.tensor_scalar_mul(gsc[:], rstd[:], gam_sb[:])
            gbias = stats.tile([128, 4], F32)
            nc.vector.tensor_mul(gbias[:], gstat[:, 0:4], gsc[:])
            nc.vector.tensor_scalar(
                out=gbias[:], in0=gbias[:], scalar1=-1.0, scalar2=bet_sb[:],
                op0=mybir.AluOpType.mult, op1=mybir.AluOpType.add,
            )
            # xn = x * gsc + gbias: fp32 in place (residual path) and rounded
            # f32r copy (matmul path)
            for r in range(4):
                nc.vector.tensor_scalar(
                    out=x_sb[:, 512 * r : 512 * r + 512],
                    in0=x_sb[:, 512 * r : 512 * r + 512],
                    scalar1=gsc[:, r : r + 1], scalar2=gbias[:, r : r + 1],
                    op0=mybir.AluOpType.mult, op1=mybir.AluOpType.add,
                )
            nc.vector.tensor_copy(xn_r[:], x_sb[:])

            # ---- Q/K/V projections ----
            # q^T duplicated on both partition halves: lhsT = [Wq | Wq]
            for t in range(4):
                sl = slice(512 * t, 512 * t + 512)
                ps = pre_ps.tile([128, 512], F32, tag="pre")
                nc.tensor.matmul(
                    ps[:], bq_sb[:], ones_sb[0:1, :], start=True, stop=False
                )
                nc.tensor.matmul(
                    ps[:], wq_sb[:], xn_r[0:64, sl], start=False, stop=True
                )
                nc.vector.tensor_copy(q0_sb[0:64, sl], ps[0:64, :])
                nc.vector.tensor_copy(q1_sb[64:128, sl], ps[64:128, :])
            # k^T packed by half: lhsT = blockdiag(Wk, Wk)
            for t in range(4):
                sl = slice(512 * t, 512 * t + 512)
                ps = pre_ps.tile([128, 512], F32, tag="pre")
                nc.tensor.matmul(
                    ps[:], bk_sb[:], ones_sb[0:1, :], start=True, stop=False
                )
                nc.tensor.matmul(
                    ps[:], wk_sb[:], xn_r[:, sl], start=False, stop=True
                )
                nc.vector.tensor_copy(kt_sb[:, sl], ps[:])
            # v position-major, two chunks per matmul: out[pos, (half, c)]
            for u in range(16):
                sl = slice(128 * u, 128 * u + 128)
                ps = pre_ps.tile([128, 128], F32, tag="pre")
                nc.tensor.matmul(
                    ps[:], ones_sb[0:1, 0:128], bv_sb[:], start=True, stop=False
                )
                nc.tensor.matmul(
                    ps[:], xn_r[:, sl], wv_sb[:], start=False, stop=True
                )
                nc.vector.tensor_copy(v_all[:, 65 * u : 65 * u + 64], ps[:, 0:64])
                nc.vector.tensor_copy(
                    v_all[:, 65 * (u + 16) : 65 * (u + 16) + 64], ps[:, 64:128]
                )

        with tc.tile_pool(name="sc_ps", bufs=2, space="PSUM") as sc_ps, \
             tc.tile_pool(name="pj_ps", bufs=2, space="PSUM") as pj_ps, \
             tc.tile_pool(name="work", bufs=2) as work:

            batches = [list(range(t0, min(t0 + EXP_BATCH, 32)))
                       for t0 in range(0, 32, EXP_BATCH)]
            attn_bufs = [attnexp_a, attnexp_b]
            paccs = [None, None, None, None]

            def emit_scores_batch(n, batch):
                attnexp = attn_bufs[n % 2]
                qsl = slice(512 * n, 512 * n + 512)
                nb = len(batch)
                ps = sc_ps.tile([128, 512 * EXP_BATCH], F32, tag="sc")
                for i, t in enumerate(batch):
                    lhsT = kt_sb[:, 128 * (t % 16) : 128 * (t % 16) + 128]
                    rhs = (q0_sb if t < 16 else q1_sb)[:, qsl]
                    nc.tensor.matmul(
                        ps[:, 512 * i : 512 * i + 512], lhsT, rhs,
                        start=True, stop=True,
                    )
                off = 512 * batch[0]
                nc.scalar.activation(
                    attnexp[:, off : off + 512 * nb], ps[:, 0 : 512 * nb],
                    mybir.ActivationFunctionType.Exp, scale=SCALE,
                )

            def emit_proj_batch(n, batch):
                attnexp = attn_bufs[n % 2]
                if paccs[n] is None:
                    paccs[n] = pj_ps.tile([65, 512], F32, tag="pj", name=f"pacc{n}")
                pacc = paccs[n]
                for t in batch:
                    nc.tensor.matmul(
                        pacc[:], v_all[:, 65 * t : 65 * t + 65],
                        attnexp[:, 512 * t : 512 * t + 512],
                        start=(t == 0), stop=(t == 31),
                    )

            def emit_finish_a(n):
                # free the PSUM accumulator ASAP, then compute 1/denom on DVE
                # off the PE critical path
                pacc = paccs[n]
                pacc_sb = work.tile([65, 512], F32, tag="pacc_sb", name=f"psb{n}")
                nc.vector.tensor_copy(pacc_sb[:], pacc[:])
                rd = work.tile([65, 512], MDT, tag="rd", name=f"rd{n}")
                with nc.allow_low_precision(reason="softmax denom rounding"):
                    nc.vector.reciprocal(rd[64:65, :], pacc_sb[64:65, :])
                return pacc_sb, rd

            def emit_finish_b(n, pacc_sb, rd):
                # broadcast 1/denom across partitions (tiny PE matmul),
                # normalize, out-projection, bias + residual
                qsl = slice(512 * n, 512 * n + 512)
                bc_ps = pj_ps.tile([64, 512], F32, tag="pj", name=f"bc{n}")
                nc.tensor.matmul(
                    bc_ps[:], ones_sb[64:65, 0:64], rd[64:65, :], start=True,
                    stop=True
                )
                bc_sb = work.tile([64, 512], F32, tag="bc", name=f"bcs{n}")
                nc.vector.tensor_copy(bc_sb[:], bc_ps[:])
                projn = work.tile([64, 512], MDT, tag="projn", name=f"pn{n}")
                nc.vector.tensor_mul(projn[:], pacc_sb[0:64, :], bc_sb[:])
                fps = pj_ps.tile([64, 512], F32, tag="pj", name=f"fps{n}")
                nc.tensor.matmul(
                    fps[:], bo_sb[:], ones_sb[0:1, :], start=True, stop=False
                )
                nc.tensor.matmul(
                    fps[:], wo_sb[:], projn[:], start=False, stop=True
                )
                nc.vector.tensor_add(out_sb[:, qsl], fps[:], x_sb[0:64, qsl])

            # ---- software-pipelined attention: scores/exp of tile n
            # interleaved with attn@V of tile n-1; the previous tile's
            # normalize/out-proj ops are emitted a few batches in so the PE
            # never stalls on the softmax-denominator chain ----
            pending = None
            for n in range(4):
                for bi, batch in enumerate(batches):
                    emit_scores_batch(n, batch)
                    if n > 0:
                        emit_proj_batch(n - 1, batch)
                    if bi == 2 and pending is not None:
                        emit_finish_b(*pending)
                        pending = None
                if n > 0:
                    pending = (n - 1,) + emit_finish_a(n - 1)
            for batch in batches:
                emit_proj_batch(3, batch)
                if pending is not None and batch[0] == batches[2][0]:
                    emit_finish_b(*pending)
                    pending = None
            emit_finish_b(3, *emit_finish_a(3))
        nc.sync.dma_start(out_d.ap(), out_sb[:])

    nc.compile()
    return nc


def host_prep(x, gamma, beta, Wq, bq, Wk, bk, Wv, bv, Wo, bo):
    """Build the 8 per-core input dicts."""
    f32 = lambda a: np.ascontiguousarray(np.asarray(a, np.float32))
    x = f32(x)
    gamma, beta = f32(gamma), f32(beta)
    Wq, Wk, Wv, Wo = f32(Wq), f32(Wk), f32(Wv), f32(Wo)
    bq, bk, bv, bo = f32(bq), f32(bk), f32(bv), f32(bo)

    wq_dup = np.ascontiguousarray(np.concatenate([Wq, Wq], axis=1))
    z = np.zeros((64, 64), np.float32)
    wk_blk = np.ascontiguousarray(np.block([[Wk, z], [z, Wk]]))
    wv_blk = np.ascontiguousarray(np.block([[Wv, z], [z, Wv]]))
    comb = np.zeros((128, 128), np.float32)
    comb[:64, :64] = 1.0 / 64.0
    comb[64:, 64:] = 1.0 / 64.0
    mdt_np = mybir.dt.np(MDT)
    m = lambda a: np.ascontiguousarray(a).astype(mdt_np)
    shared = {
        "wq": m(wq_dup), "wk": m(wk_blk), "wv": m(wv_blk), "wo": m(Wo),
        "bq": m(np.tile(bq, 2)[None]),
        "bk": m(np.tile(bk, 2)[None]),
        "bv": m(np.tile(bv, 2)[None]),
        "bo": m(bo[None]),
        "gam": np.ascontiguousarray(np.tile(gamma, 2)[:, None]),
        "bet": np.ascontiguousarray(np.tile(beta, 2)[:, None]),
        "comb": comb,
    }
    in_maps = []
    for core in range(8):
        b, h = core // 2, core % 2
        xT = x[b].reshape(HW, C).T  # [64, 4096]
        halves = xT.reshape(C, 2, HALF)[:, [h, 1 - h], :]
        xp = np.ascontiguousarray(halves.transpose(1, 0, 2).reshape(128, HALF))
        in_maps.append({"xp": xp, **shared})
    return in_maps


def assemble(results, dtype):
    out = np.empty((B, HW, C), np.float32)
    for core in range(8):
        b, h = core // 2, core % 2
        out[b, HALF * h : HALF * h + HALF] = results[core]["out"].T
    return out.reshape(B, H, W, C).astype(dtype, copy=False)


_NC_CACHE = []


def kernel(x, gamma, beta, Wq, bq, Wk, bk, Wv, bv, Wo, bo):
    from concourse.bass_utils import run_bass_kernel_spmd

    if not _NC_CACHE:
        _NC_CACHE.append(build_nc())
    nc = _NC_CACHE[0]
    in_maps = host_prep(x, gamma, beta, Wq, bq, Wk, bk, Wv, bv, Wo, bo)
    res = run_bass_kernel_spmd(nc, in_maps, core_ids=list(range(8)))
    return assemble(res.results, np.asarray(x).dtype)


if __name__ == "__main__":
    rng = np.random.default_rng(0)
    inputs = {
        "x": rng.standard_normal((B, H, W, C)).astype(np.float32),
        "gamma": np.ones(C, np.float32), "beta": np.zeros(C, np.float32),
        "Wq": (rng.standard_normal((C, C)) / 8).astype(np.float32),
        "bq": np.zeros(C, np.float32),
        "Wk": (rng.standard_normal((C, C)) / 8).astype(np.float32),
        "bk": np.zeros(C, np.float32),
        "Wv": (rng.standard_normal((C, C)) / 8).astype(np.float32),
        "bv": np.zeros(C, np.float32),
        "Wo": (rng.standard_normal((C, C)) / 8).astype(np.float32),
        "bo": np.zeros(C, np.float32),
    }
    out = kernel(**inputs)
    print("kernel ran, out shape", out.shape, out.dtype)


# revision 13
# speedup vs baseline: 2.9346x; 1.0657x over previous
"""Trainium2 Bass kernel for nn_AttentionBlock (B=4, H=W=64, C=64, GroupNorm(8) +
full spatial self-attention), distributed over 8 NeuronCores.

Sharding: core i handles batch b=i//2 and query-half h=i%2 (2048 of the 4096
spatial positions). Each core computes the full GroupNorm and K/V for its
image (cheap) and attention only for its query half. No collectives.

Device layout: channel-on-partition ("xT") layout, with the two position
halves of an image packed onto partitions [(half, channel)] -> 128 partitions.
Scores are computed transposed (positions' on partitions) so that
- softmax reduction over positions' is a PE matmul (ones column in V), and
- the attention @ V contraction needs no transposes at all.
exp() runs on ScalarE reading PSUM and writing SBUF directly.

PE matmuls run in float32r (1 cycle/row at N>=256, ~tf32 mantissa); GroupNorm
statistics and the residual path stay full fp32.
"""

import sys

sys.path.insert(0, "/opt/trn_rl_repo")

import numpy as np

import concourse.bacc as bacc
import concourse.tile as tile
from concourse import mybir

B, H, W, C = 4, 64, 64, 64
HW = H * W  # 4096
HALF = HW // 2  # 2048
EPS = 1e-5
SCALE = C ** -0.5  # folded into exp()

F32 = mybir.dt.float32
# dtype of the PE matmul operands: float32r runs at 4x the float32 rate.
MDT = mybir.dt.bfloat16

EXP_BATCH = 3  # pos'-chunks (PSUM banks) per exp() call


def build_nc():
    nc = bacc.Bacc("TRN2", debug=False, num_devices=8)

    # ---- DRAM I/O ----
    xp_d = nc.dram_tensor("xp", [128, HALF], F32, kind="ExternalInput")
    wq_d = nc.dram_tensor("wq", [64, 128], MDT, kind="ExternalInput")
    wk_d = nc.dram_tensor("wk", [128, 128], MDT, kind="ExternalInput")
    wv_d = nc.dram_tensor("wv", [128, 128], MDT, kind="ExternalInput")
    wo_d = nc.dram_tensor("wo", [64, 64], MDT, kind="ExternalInput")
    bq_d = nc.dram_tensor("bq", [1, 128], MDT, kind="ExternalInput")
    bk_d = nc.dram_tensor("bk", [1, 128], MDT, kind="ExternalInput")
    bv_d = nc.dram_tensor("bv", [1, 128], MDT, kind="ExternalInput")
    bo_d = nc.dram_tensor("bo", [1, 64], MDT, kind="ExternalInput")
    gam_d = nc.dram_tensor("gam", [128, 1], F32, kind="ExternalInput")
    bet_d = nc.dram_tensor("bet", [128, 1], F32, kind="ExternalInput")
    comb_d = nc.dram_tensor("comb", [128, 128], F32, kind="ExternalInput")
    out_d = nc.dram_tensor("out", [64, HALF], F32, kind="ExternalOutput")

    with tile.TileContext(nc) as tc, \
         tc.tile_pool(name="singles", bufs=1) as singles, \
         tc.tile_pool(name="stats", bufs=4) as stats:

        # ---- constant loads ----
        x_sb = singles.tile([128, HALF], F32)
        for r in range(4):
            nc.sync.dma_start(
                x_sb[:, 512 * r : 512 * r + 512],
                xp_d.ap()[:, 512 * r : 512 * r + 512],
            )
        wq_sb = singles.tile([64, 128], MDT)
        nc.sync.dma_start(wq_sb[:], wq_d.ap())
        wk_sb = singles.tile([128, 128], MDT)
        nc.sync.dma_start(wk_sb[:], wk_d.ap())
        wv_sb = singles.tile([128, 128], MDT)
        nc.sync.dma_start(wv_sb[:], wv_d.ap())
        wo_sb = singles.tile([64, 64], MDT)
        nc.sync.dma_start(wo_sb[:], wo_d.ap())
        bq_sb = singles.tile([1, 128], MDT)
        nc.sync.dma_start(bq_sb[:], bq_d.ap())
        bk_sb = singles.tile([1, 128], MDT)
        nc.sync.dma_start(bk_sb[:], bk_d.ap())
        bv_sb = singles.tile([1, 128], MDT)
        nc.sync.dma_start(bv_sb[:], bv_d.ap())
        bo_sb = singles.tile([1, 64], MDT)
        nc.sync.dma_start(bo_sb[:], bo_d.ap())
        gam_sb = singles.tile([128, 1], F32)
        nc.sync.dma_start(gam_sb[:], gam_d.ap())
        bet_sb = singles.tile([128, 1], F32)
        nc.sync.dma_start(bet_sb[:], bet_d.ap())
        comb_sb = singles.tile([128, 128], F32)
        nc.sync.dma_start(comb_sb[:], comb_d.ap())

        ones_f = singles.tile([128, 512], F32)
        nc.vector.memset(ones_f[:], 1.0)
        ones_sb = singles.tile([128, 512], MDT)
        nc.vector.tensor_copy(ones_sb[:], ones_f[:])

        xn_r = singles.tile([128, HALF], MDT)
        q0_sb = singles.tile([128, HALF], MDT)
        q1_sb = singles.tile([128, HALF], MDT)
        kt_sb = singles.tile([128, HALF], MDT)
        v_all = singles.tile([128, 65 * 32], MDT)
        attnexp_a = singles.tile([128, 512 * 32], MDT)
        attnexp_b = singles.tile([128, 512 * 32], MDT)
        out_sb = singles.tile([64, HALF], F32)

        # zero halves of the padded-query operands (once)
        zeros_f = singles.tile([64, HALF], F32)
        nc.vector.memset(zeros_f[:], 0.0)
        nc.vector.tensor_copy(q0_sb[64:128, :], zeros_f[:])
        nc.vector.tensor_copy(q1_sb[0:64, :], zeros_f[:])

        # ones column of v_all (chunk t's column 64 -> softmax denominator)
        v3 = v_all[:].rearrange("p (t e) -> p t e", e=65)
        nc.vector.tensor_copy(
            v3[:, :, 64:65], ones_f[:, 0:32].rearrange("p (a b) -> p a b", b=1)
        )

        # ---- GroupNorm (stats per partition per 512-slice, then a
        # block-diagonal averaging matmul combines across channels) ----
        smat = stats.tile([128, 8], F32)  # cols 0-3 mean, 4-7 E[x^2]
        for r in range(4):
            st6 = stats.tile([128, 6], F32, tag="st6")
            nc.vector.bn_stats(st6[:], x_sb[:, 512 * r : 512 * r + 512])
            mv = stats.tile([128, 2], F32, tag="mv")
            nc.vector.bn_aggr(mv[:], st6[:])
            nc.vector.tensor_copy(smat[:, r : r + 1], mv[:, 0:1])
            sq = stats.tile([128, 1], F32, tag="sq")
            nc.vector.tensor_mul(sq[:], mv[:, 0:1], mv[:, 0:1])
            nc.vector.tensor_add(smat[:, 4 + r : 5 + r], mv[:, 1:2], sq[:])

        with tc.tile_pool(name="pre_ps", bufs=2, space="PSUM") as pre_ps:
            cps = pre_ps.tile([128, 8], F32, tag="pre")
            nc.tensor.matmul(cps[:], comb_sb[:], smat[:], start=True, stop=True)
            gstat = stats.tile([128, 8], F32)  # 0-3 mean_g, 4-7 E2_g
            nc.vector.tensor_copy(gstat[:], cps[:])

            var_g = stats.tile([128, 4], F32)
            nc.vector.tensor_mul(var_g[:], gstat[:, 0:4], gstat[:, 0:4])
            nc.vector.tensor_tensor(
                var_g[:], gstat[:, 4:8], var_g[:], op=mybir.AluOpType.subtract
            )
            # rstd = exp(-0.5 * ln(var + eps)); Ln/Exp share one ACT table set
            eps_sb = stats.tile([128, 1], F32)
            nc.vector.memset(eps_sb[:], EPS)
            rstd = stats.tile([128, 4], F32)
            nc.scalar.activation(
                rstd[:], var_g[:], mybir.ActivationFunctionType.Ln, bias=eps_sb[:]
            )
            nc.scalar.activation(
                rstd[:], rstd[:], mybir.ActivationFunctionType.Exp, scale=-0.5
            )
            gsc = stats.tile([128, 4], F32)
            nc.vector.tensor_scalar_mul(gsc[:], rstd[:], gam_sb[:])
            gbias = stats.tile([128, 4], F32)
            nc.vector.tensor_mul(gbias[:], gstat[:, 0:4], gsc[:])
            nc.vector.tensor_scalar(
                out=gbias[:], in0=gbias[:], scalar1=-1.0, scalar2=bet_sb[:],
                op0=mybir.AluOpType.mult, op1=mybir.AluOpType.add,
            )
            # xn = x * gsc + gbias: bf16 copy for the matmuls via ScalarE
            # (runs parallel to the DVE fp32 pass used by the residual path)
            for r in range(4):
                sl = slice(512 * r, 512 * r + 512)
                nc.scalar.activation(
                    xn_r[:, sl], x_sb[:, sl],
                    mybir.ActivationFunctionType.Identity,
                    bias=gbias[:, r : r + 1], scale=gsc[:, r : r + 1],
                )
                nc.vector.tensor_scalar(
                    out=x_sb[:, sl], in0=x_sb[:, sl],
                    scalar1=gsc[:, r : r + 1], scalar2=gbias[:, r : r + 1],
                    op0=mybir.AluOpType.mult, op1=mybir.AluOpType.add,
                )

            # ---- Q/K/V projections ----
            # q^T duplicated on both partition halves: lhsT = [Wq | Wq]
            for t in range(4):
                sl = slice(512 * t, 512 * t + 512)
                ps = pre_ps.tile([128, 512], F32, tag="pre")
                nc.tensor.matmul(
                    ps[:], bq_sb[:], ones_sb[0:1, :], start=True, stop=False
                )
                nc.tensor.matmul(
                    ps[:], wq_sb[:], xn_r[0:64, sl], start=False, stop=True
                )
                nc.vector.tensor_copy(q0_sb[0:64, sl], ps[0:64, :])
                nc.vector.tensor_copy(q1_sb[64:128, sl], ps[64:128, :])
            # k^T packed by half: lhsT = blockdiag(Wk, Wk)
            for t in range(4):
                sl = slice(512 * t, 512 * t + 512)
                ps = pre_ps.tile([128, 512], F32, tag="pre")
                nc.tensor.matmul(
                    ps[:], bk_sb[:], ones_sb[0:1, :], start=True, stop=False
                )
                nc.tensor.matmul(
                    ps[:], wk_sb[:], xn_r[:, sl], start=False, stop=True
                )
                nc.vector.tensor_copy(kt_sb[:, sl], ps[:])

        with tc.tile_pool(name="sc_ps", bufs=2, space="PSUM") as sc_ps, \
             tc.tile_pool(name="pj_ps", bufs=2, space="PSUM") as pj_ps, \
             tc.tile_pool(name="work", bufs=2) as work:

            batches = [list(range(t0, min(t0 + EXP_BATCH, 32)))
                       for t0 in range(0, 32, EXP_BATCH)]
            attn_bufs = [attnexp_a, attnexp_b]
            paccs = [None, None, None, None]

            def emit_v_pair(u):
                # v position-major, two chunks per matmul: out[pos, (half, c)]
                sl = slice(128 * u, 128 * u + 128)
                ps = pj_ps.tile([128, 128], F32, tag="pj", name=f"vps{u}")
                nc.tensor.matmul(
                    ps[:], ones_sb[0:1, 0:128], bv_sb[:], start=True, stop=False
                )
                nc.tensor.matmul(
                    ps[:], xn_r[:, sl], wv_sb[:], start=False, stop=True
                )
                nc.vector.tensor_copy(v_all[:, 65 * u : 65 * u + 64], ps[:, 0:64])
                nc.vector.tensor_copy(
                    v_all[:, 65 * (u + 16) : 65 * (u + 16) + 64], ps[:, 64:128]
                )

            def emit_scores_batch(n, batch):
                attnexp = attn_bufs[n % 2]
                qsl = slice(512 * n, 512 * n + 512)
                nb = len(batch)
                ps = sc_ps.tile([128, 512 * EXP_BATCH], F32, tag="sc")
                for i, t in enumerate(batch):
                    lhsT = kt_sb[:, 128 * (t % 16) : 128 * (t % 16) + 128]
                    rhs = (q0_sb if t < 16 else q1_sb)[:, qsl]
                    nc.tensor.matmul(
                        ps[:, 512 * i : 512 * i + 512], lhsT, rhs,
                        start=True, stop=True,
                    )
                off = 512 * batch[0]
                nc.scalar.activation(
                    attnexp[:, off : off + 512 * nb], ps[:, 0 : 512 * nb],
                    mybir.ActivationFunctionType.Exp, scale=SCALE,
                )

            def emit_proj_batch(n, batch):
                attnexp = attn_bufs[n % 2]
                if paccs[n] is None:
                    paccs[n] = pj_ps.tile([65, 512], F32, tag="pj", name=f"pacc{n}")
                pacc = paccs[n]
                for t in batch:
                    nc.tensor.matmul(
                        pacc[:], v_all[:, 65 * t : 65 * t + 65],
                        attnexp[:, 512 * t : 512 * t + 512],
                        start=(t == 0), stop=(t == 31),
                    )

            def emit_finish_a(n):
                # free the PSUM accumulator ASAP, then compute 1/denom on DVE
                # off the PE critical path
                pacc = paccs[n]
                pacc_sb = work.tile([65, 512], F32, tag="pacc_sb", name=f"psb{n}")
                nc.vector.tensor_copy(pacc_sb[:], pacc[:])
                rd = work.tile([65, 512], MDT, tag="rd", name=f"rd{n}")
                with nc.allow_low_precision(reason="softmax denom rounding"):
                    nc.vector.reciprocal(rd[64:65, :], pacc_sb[64:65, :])
                return pacc_sb, rd

            def emit_finish_b(n, pacc_sb, rd):
                # broadcast 1/denom across partitions (tiny PE matmul),
                # normalize, out-projection, bias + residual
                qsl = slice(512 * n, 512 * n + 512)
                bc_ps = pj_ps.tile([64, 512], F32, tag="pj", name=f"bc{n}")
                nc.tensor.matmul(
                    bc_ps[:], ones_sb[64:65, 0:64], rd[64:65, :], start=True,
                    stop=True
                )
                bc_sb = work.tile([64, 512], F32, tag="bc", name=f"bcs{n}")
                nc.vector.tensor_copy(bc_sb[:], bc_ps[:])
                projn = work.tile([64, 512], MDT, tag="projn", name=f"pn{n}")
                nc.vector.tensor_mul(projn[:], pacc_sb[0:64, :], bc_sb[:])
                fps = pj_ps.tile([64, 512], F32, tag="pj", name=f"fps{n}")
                nc.tensor.matmul(
                    fps[:], bo_sb[:], ones_sb[0:1, :], start=True, stop=False
                )
                nc.tensor.matmul(
                    fps[:], wo_sb[:], projn[:], start=False, stop=True
                )
                nc.vector.tensor_add(out_sb[:, qsl], fps[:], x_sb[0:64, qsl])
                nc.sync.dma_start(out_d.ap()[:, qsl], out_sb[:, qsl])

            # ---- software-pipelined attention: scores/exp of tile n
            # interleaved with attn@V of tile n-1; the previous tile's
            # normalize/out-proj ops are emitted a few batches in so the PE
            # never stalls on the softmax-denominator chain ----
            pending = None
            for n in range(4):
                for bi, batch in enumerate(batches):
                    emit_scores_batch(n, batch)
                    if n == 0 and bi < 8:
                        emit_v_pair(2 * bi)
                        emit_v_pair(2 * bi + 1)
                    if n > 0:
                        emit_proj_batch(n - 1, batch)
                    if bi == 2 and pending is not None:
                        emit_finish_b(*pending)
                        pending = None
                if n > 0:
                    pending = (n - 1,) + emit_finish_a(n - 1)
            for batch in batches:
                emit_proj_batch(3, batch)
                if pending is not None and batch[0] == batches[2][0]:
                    emit_finish_b(*pending)
                    pending = None
            emit_finish_b(3, *emit_finish_a(3))

    nc.compile()
    return nc


def host_prep(x, gamma, beta, Wq, bq, Wk, bk, Wv, bv, Wo, bo):
    """Build the 8 per-core input dicts."""
    f32 = lambda a: np.ascontiguousarray(np.asarray(a, np.float32))
    x = f32(x)
    gamma, beta = f32(gamma), f32(beta)
    Wq, Wk, Wv, Wo = f32(Wq), f32(Wk), f32(Wv), f32(Wo)
    bq, bk, bv, bo = f32(bq), f32(bk), f32(bv), f32(bo)

    wq_dup = np.ascontiguousarray(np.concatenate([Wq, Wq], axis=1))
    z = np.zeros((64, 64), np.float32)
    wk_blk = np.ascontiguousarray(np.block([[Wk, z], [z, Wk]]))
    wv_blk = np.ascontiguousarray(np.block([[Wv, z], [z, Wv]]))
    comb = np.zeros((128, 128), np.float32)
    comb[:64, :64] = 1.0 / 64.0
    comb[64:, 64:] = 1.0 / 64.0
    mdt_np = mybir.dt.np(MDT)
    m = lambda a: np.ascontiguousarray(a).astype(mdt_np)
    shared = {
        "wq": m(wq_dup), "wk": m(wk_blk), "wv": m(wv_blk), "wo": m(Wo),
        "bq": m(np.tile(bq, 2)[None]),
        "bk": m(np.tile(bk, 2)[None]),
        "bv": m(np.tile(bv, 2)[None]),
        "bo": m(bo[None]),
        "gam": np.ascontiguousarray(np.tile(gamma, 2)[:, None]),
        "bet": np.ascontiguousarray(np.tile(beta, 2)[:, None]),
        "comb": comb,
    }
    in_maps = []
    for core in range(8):
        b, h = core // 2, core % 2
        xT = x[b].reshape(HW, C).T  # [64, 4096]
        halves = xT.reshape(C, 2, HALF)[:, [h, 1 - h], :]
        xp = np.ascontiguousarray(halves.transpose(1, 0, 2).reshape(128, HALF))
        in_maps.append({"xp": xp, **shared})
    return in_maps


def assemble(results, dtype):
    out = np.empty((B, HW, C), np.float32)
    for core in range(8):
        b, h = core // 2, core % 2
        out[b, HALF * h : HALF * h + HALF] = results[core]["out"].T
    return out.reshape(B, H, W, C).astype(dtype, copy=False)


_NC_CACHE = []


def kernel(x, gamma, beta, Wq, bq, Wk, bk, Wv, bv, Wo, bo):
    from concourse.bass_utils import run_bass_kernel_spmd

    if not _NC_CACHE:
        _NC_CACHE.append(build_nc())
    nc = _NC_CACHE[0]
    in_maps = host_prep(x, gamma, beta, Wq, bq, Wk, bk, Wv, bv, Wo, bo)
    res = run_bass_kernel_spmd(nc, in_maps, core_ids=list(range(8)))
    return assemble(res.results, np.asarray(x).dtype)


if __name__ == "__main__":
    rng = np.random.default_rng(0)
    inputs = {
        "x": rng.standard_normal((B, H, W, C)).astype(np.float32),
        "gamma": np.ones(C, np.float32), "beta": np.zeros(C, np.float32),
        "Wq": (rng.standard_normal((C, C)) / 8).astype(np.float32),
        "bq": np.zeros(C, np.float32),
        "Wk": (rng.standard_normal((C, C)) / 8).astype(np.float32),
        "bk": np.zeros(C, np.float32),
        "Wv": (rng.standard_normal((C, C)) / 8).astype(np.float32),
        "bv": np.zeros(C, np.float32),
        "Wo": (rng.standard_normal((C, C)) / 8).astype(np.float32),
        "bo": np.zeros(C, np.float32),
    }
    out = kernel(**inputs)
    print("kernel ran, out shape", out.shape, out.dtype)


# revision 14
# speedup vs baseline: 2.9791x; 1.0151x over previous
"""Trainium2 Bass kernel for nn_AttentionBlock (B=4, H=W=64, C=64, GroupNorm(8) +
full spatial self-attention), distributed over 8 NeuronCores.

Sharding: core i handles batch b=i//2 and query-half h=i%2 (2048 of the 4096
spatial positions). Each core computes the full GroupNorm and K/V for its
image (cheap) and attention only for its query half. No collectives.

Device layout: channel-on-partition ("xT") layout, with the two position
halves of an image packed onto partitions [(half, channel)] -> 128 partitions.
Scores are computed transposed (positions' on partitions) so that
- softmax reduction over positions' is a PE matmul (ones column in V), and
- the attention @ V contraction needs no transposes at all.
exp() runs on ScalarE reading PSUM and writing SBUF directly.

PE matmuls run in float32r (1 cycle/row at N>=256, ~tf32 mantissa); GroupNorm
statistics and the residual path stay full fp32.
"""

import sys

sys.path.insert(0, "/opt/trn_rl_repo")

import numpy as np

import concourse.bacc as bacc
import concourse.tile as tile
from concourse import mybir

B, H, W, C = 4, 64, 64, 64
HW = H * W  # 4096
HALF = HW // 2  # 2048
EPS = 1e-5
SCALE = C ** -0.5  # folded into exp()

F32 = mybir.dt.float32
# dtype of the PE matmul operands: float32r runs at 4x the float32 rate.
MDT = mybir.dt.bfloat16

EXP_BATCH = 3  # pos'-chunks (PSUM banks) per exp() call


def build_nc():
    nc = bacc.Bacc("TRN2", debug=False, num_devices=8)

    # ---- DRAM I/O ----
    xp_d = nc.dram_tensor("xp", [128, HALF], F32, kind="ExternalInput")
    wq_d = nc.dram_tensor("wq", [64, 128], MDT, kind="ExternalInput")
    wk_d = nc.dram_tensor("wk", [128, 128], MDT, kind="ExternalInput")
    wv_d = nc.dram_tensor("wv", [128, 128], MDT, kind="ExternalInput")
    wo_d = nc.dram_tensor("wo", [64, 64], MDT, kind="ExternalInput")
    bq_d = nc.dram_tensor("bq", [1, 128], MDT, kind="ExternalInput")
    bk_d = nc.dram_tensor("bk", [1, 128], MDT, kind="ExternalInput")
    bv_d = nc.dram_tensor("bv", [1, 128], MDT, kind="ExternalInput")
    bo_d = nc.dram_tensor("bo", [1, 64], MDT, kind="ExternalInput")
    gam_d = nc.dram_tensor("gam", [128, 1], F32, kind="ExternalInput")
    bet_d = nc.dram_tensor("bet", [128, 1], F32, kind="ExternalInput")
    comb_d = nc.dram_tensor("comb", [128, 128], F32, kind="ExternalInput")
    out_d = nc.dram_tensor("out", [64, HALF], F32, kind="ExternalOutput")

    with tile.TileContext(nc) as tc, \
         tc.tile_pool(name="singles", bufs=1) as singles, \
         tc.tile_pool(name="stats", bufs=4) as stats:

        # ---- constant loads ----
        x_sb = singles.tile([128, HALF], F32)
        for r in range(4):
            nc.sync.dma_start(
                x_sb[:, 512 * r : 512 * r + 512],
                xp_d.ap()[:, 512 * r : 512 * r + 512],
            )
        wq_sb = singles.tile([64, 128], MDT)
        nc.gpsimd.dma_start(wq_sb[:], wq_d.ap())
        wk_sb = singles.tile([128, 128], MDT)
        nc.gpsimd.dma_start(wk_sb[:], wk_d.ap())
        wv_sb = singles.tile([128, 128], MDT)
        nc.gpsimd.dma_start(wv_sb[:], wv_d.ap())
        wo_sb = singles.tile([64, 64], MDT)
        nc.gpsimd.dma_start(wo_sb[:], wo_d.ap())
        bq_sb = singles.tile([1, 128], MDT)
        nc.gpsimd.dma_start(bq_sb[:], bq_d.ap())
        bk_sb = singles.tile([1, 128], MDT)
        nc.gpsimd.dma_start(bk_sb[:], bk_d.ap())
        bv_sb = singles.tile([1, 128], MDT)
        nc.gpsimd.dma_start(bv_sb[:], bv_d.ap())
        bo_sb = singles.tile([1, 64], MDT)
        nc.gpsimd.dma_start(bo_sb[:], bo_d.ap())
        gam_sb = singles.tile([128, 1], F32)
        nc.gpsimd.dma_start(gam_sb[:], gam_d.ap())
        bet_sb = singles.tile([128, 1], F32)
        nc.gpsimd.dma_start(bet_sb[:], bet_d.ap())
        comb_sb = singles.tile([128, 128], F32)
        nc.gpsimd.dma_start(comb_sb[:], comb_d.ap())

        ones_f = singles.tile([128, 512], F32)
        nc.vector.memset(ones_f[:], 1.0)
        scr = singles.tile([128, 1], F32)
        nc.scalar.activation(scr[:], ones_f[:, 0:1],
                             mybir.ActivationFunctionType.Exp)
        ones_sb = singles.tile([128, 512], MDT)
        nc.vector.tensor_copy(ones_sb[:], ones_f[:])

        xn_r = singles.tile([128, HALF], MDT)
        q0_sb = singles.tile([128, HALF], MDT)
        q1_sb = singles.tile([128, HALF], MDT)
        kt_sb = singles.tile([128, HALF], MDT)
        v_all = singles.tile([128, 65 * 32], MDT)
        attnexp_a = singles.tile([128, 512 * 32], MDT)
        attnexp_b = singles.tile([128, 512 * 32], MDT)
        out_sb = singles.tile([64, HALF], F32)

        # zero halves of the padded-query operands (once)
        zeros_f = singles.tile([64, HALF], F32)
        nc.vector.memset(zeros_f[:], 0.0)
        nc.vector.tensor_copy(q0_sb[64:128, :], zeros_f[:])
        nc.vector.tensor_copy(q1_sb[0:64, :], zeros_f[:])

        # ones column of v_all (chunk t's column 64 -> softmax denominator)
        v3 = v_all[:].rearrange("p (t e) -> p t e", e=65)
        nc.vector.tensor_copy(
            v3[:, :, 64:65], ones_f[:, 0:32].rearrange("p (a b) -> p a b", b=1)
        )

        # ---- GroupNorm (stats per partition per 512-slice, then a
        # block-diagonal averaging matmul combines across channels) ----
        smat = stats.tile([128, 8], F32)  # cols 0-3 mean, 4-7 E[x^2]
        for r in range(4):
            st6 = stats.tile([128, 6], F32, tag="st6")
            nc.vector.bn_stats(st6[:], x_sb[:, 512 * r : 512 * r + 512])
            mv = stats.tile([128, 2], F32, tag="mv")
            nc.vector.bn_aggr(mv[:], st6[:])
            nc.vector.tensor_copy(smat[:, r : r + 1], mv[:, 0:1])
            sq = stats.tile([128, 1], F32, tag="sq")
            nc.vector.tensor_mul(sq[:], mv[:, 0:1], mv[:, 0:1])
            nc.vector.tensor_add(smat[:, 4 + r : 5 + r], mv[:, 1:2], sq[:])

        with tc.tile_pool(name="pre_ps", bufs=4, space="PSUM") as pre_ps:
            cps = pre_ps.tile([128, 8], F32, tag="pre")
            nc.tensor.matmul(cps[:], comb_sb[:], smat[:], start=True, stop=True)
            gstat = stats.tile([128, 8], F32)  # 0-3 mean_g, 4-7 E2_g
            nc.vector.tensor_copy(gstat[:], cps[:])

            var_g = stats.tile([128, 4], F32)
            nc.vector.tensor_mul(var_g[:], gstat[:, 0:4], gstat[:, 0:4])
            nc.vector.tensor_tensor(
                var_g[:], gstat[:, 4:8], var_g[:], op=mybir.AluOpType.subtract
            )
            # rstd = rsqrt(var + eps): bit-trick seed + 3 Newton iterations
            # on DVE (keeps ACT tables free for exp)
            eps_sb = stats.tile([128, 1], F32)
            nc.vector.memset(eps_sb[:], EPS)
            ve = stats.tile([128, 4], F32)
            nc.vector.tensor_scalar_add(ve[:], var_g[:], eps_sb[:])
            yi = stats.tile([128, 4], mybir.dt.int32)
            nc.vector.tensor_scalar(
                out=yi[:], in0=ve[:].bitcast(mybir.dt.int32), scalar1=1,
                scalar2=None, op0=mybir.AluOpType.logical_shift_right,
            )
            nc.vector.tensor_scalar(
                out=yi[:], in0=yi[:], scalar1=-1, scalar2=0x5F3759DF,
                op0=mybir.AluOpType.mult, op1=mybir.AluOpType.add,
            )
            rstd = stats.tile([128, 4], F32)
            nc.vector.tensor_copy(rstd[:], yi[:].bitcast(F32))
            vh = stats.tile([128, 4], F32)
            nc.vector.tensor_scalar_mul(vh[:], ve[:], -0.5)
            t_nw = stats.tile([128, 4], F32)
            for _ in range(3):
                nc.vector.tensor_mul(t_nw[:], rstd[:], rstd[:])
                nc.vector.tensor_mul(t_nw[:], t_nw[:], vh[:])
                nc.vector.tensor_scalar(
                    out=t_nw[:], in0=t_nw[:], scalar1=1.0, scalar2=1.5,
                    op0=mybir.AluOpType.mult, op1=mybir.AluOpType.add,
                )
                nc.vector.tensor_mul(rstd[:], rstd[:], t_nw[:])
            gsc = stats.tile([128, 4], F32)
            nc.vector.tensor_scalar_mul(gsc[:], rstd[:], gam_sb[:])
            gbias = stats.tile([128, 4], F32)
            nc.vector.tensor_mul(gbias[:], gstat[:, 0:4], gsc[:])
            nc.vector.tensor_scalar(
                out=gbias[:], in0=gbias[:], scalar1=-1.0, scalar2=bet_sb[:],
                op0=mybir.AluOpType.mult, op1=mybir.AluOpType.add,
            )
            # xn = x * gsc + gbias: bf16 copy for the matmuls via ScalarE
            # (runs parallel to the DVE fp32 pass used by the residual path)
            for r in range(4):
                sl = slice(512 * r, 512 * r + 512)
                nc.scalar.activation(
                    xn_r[:, sl], x_sb[:, sl],
                    mybir.ActivationFunctionType.Identity,
                    bias=gbias[:, r : r + 1], scale=gsc[:, r : r + 1],
                )
                nc.vector.tensor_scalar(
                    out=x_sb[:, sl], in0=x_sb[:, sl],
                    scalar1=gsc[:, r : r + 1], scalar2=gbias[:, r : r + 1],
                    op0=mybir.AluOpType.mult, op1=mybir.AluOpType.add,
                )

            # ---- Q/K/V projections ----
            # q^T duplicated on both partition halves: lhsT = [Wq | Wq]
            for t in range(4):
                sl = slice(512 * t, 512 * t + 512)
                ps = pre_ps.tile([128, 512], F32, tag="pre")
                nc.tensor.matmul(
                    ps[:], bq_sb[:], ones_sb[0:1, :], start=True, stop=False
                )
                nc.tensor.matmul(
                    ps[:], wq_sb[:], xn_r[0:64, sl], start=False, stop=True
                )
                nc.scalar.copy(q0_sb[0:64, sl], ps[0:64, :])
                nc.scalar.copy(q1_sb[64:128, sl], ps[64:128, :])
            # k^T packed by half: lhsT = blockdiag(Wk, Wk)
            for t in range(4):
                sl = slice(512 * t, 512 * t + 512)
                ps = pre_ps.tile([128, 512], F32, tag="pre")
                nc.tensor.matmul(
                    ps[:], bk_sb[:], ones_sb[0:1, :], start=True, stop=False
                )
                nc.tensor.matmul(
                    ps[:], wk_sb[:], xn_r[:, sl], start=False, stop=True
                )
                nc.vector.tensor_copy(kt_sb[:, sl], ps[:])

        with tc.tile_pool(name="sc_ps", bufs=2, space="PSUM") as sc_ps, \
             tc.tile_pool(name="pj_ps", bufs=2, space="PSUM") as pj_ps, \
             tc.tile_pool(name="work", bufs=2) as work:

            batches = [list(range(t0, min(t0 + EXP_BATCH, 32)))
                       for t0 in range(0, 32, EXP_BATCH)]
            attn_bufs = [attnexp_a, attnexp_b]
            paccs = [None, None, None, None]

            def emit_v_pair(u):
                # v position-major, two chunks per matmul: out[pos, (half, c)]
                sl = slice(128 * u, 128 * u + 128)
                ps = pj_ps.tile([128, 128], F32, tag="pj", name=f"vps{u}")
                nc.tensor.matmul(
                    ps[:], ones_sb[0:1, 0:128], bv_sb[:], start=True, stop=False
                )
                nc.tensor.matmul(
                    ps[:], xn_r[:, sl], wv_sb[:], start=False, stop=True
                )
                nc.vector.tensor_copy(v_all[:, 65 * u : 65 * u + 64], ps[:, 0:64])
                nc.vector.tensor_copy(
                    v_all[:, 65 * (u + 16) : 65 * (u + 16) + 64], ps[:, 64:128]
                )

            def emit_scores_batch(n, batch):
                attnexp = attn_bufs[n % 2]
                qsl = slice(512 * n, 512 * n + 512)
                nb = len(batch)
                ps = sc_ps.tile([128, 512 * EXP_BATCH], F32, tag="sc")
                for i, t in enumerate(batch):
                    lhsT = kt_sb[:, 128 * (t % 16) : 128 * (t % 16) + 128]
                    rhs = (q0_sb if t < 16 else q1_sb)[:, qsl]
                    nc.tensor.matmul(
                        ps[:, 512 * i : 512 * i + 512], lhsT, rhs,
                        start=True, stop=True,
                    )
                off = 512 * batch[0]
                nc.scalar.activation(
                    attnexp[:, off : off + 512 * nb], ps[:, 0 : 512 * nb],
                    mybir.ActivationFunctionType.Exp, scale=SCALE,
                )

            def emit_proj_batch(n, batch):
                attnexp = attn_bufs[n % 2]
                if paccs[n] is None:
                    paccs[n] = pj_ps.tile([65, 512], F32, tag="pj", name=f"pacc{n}")
                pacc = paccs[n]
                for t in batch:
                    nc.tensor.matmul(
                        pacc[:], v_all[:, 65 * t : 65 * t + 65],
                        attnexp[:, 512 * t : 512 * t + 512],
                        start=(t == 0), stop=(t == 31),
                    )

            def emit_finish_a(n):
                # free the PSUM accumulator ASAP, then compute 1/denom on DVE
                # off the PE critical path
                pacc = paccs[n]
                pacc_sb = work.tile([65, 512], F32, tag="pacc_sb", name=f"psb{n}")
                nc.vector.tensor_copy(pacc_sb[:], pacc[:])
                rd = work.tile([65, 512], MDT, tag="rd", name=f"rd{n}")
                with nc.allow_low_precision(reason="softmax denom rounding"):
                    nc.vector.reciprocal(rd[64:65, :], pacc_sb[64:65, :])
                return pacc_sb, rd

            def emit_finish_b(n, pacc_sb, rd):
                # broadcast 1/denom across partitions (tiny PE matmul),
                # normalize, out-projection, bias + residual
                qsl = slice(512 * n, 512 * n + 512)
                bc_ps = pj_ps.tile([64, 512], F32, tag="pj", name=f"bc{n}")
                nc.tensor.matmul(
                    bc_ps[:], ones_sb[64:65, 0:64], rd[64:65, :], start=True,
                    stop=True
                )
                bc_sb = work.tile([64, 512], F32, tag="bc", name=f"bcs{n}")
                nc.vector.tensor_copy(bc_sb[:], bc_ps[:])
                projn = work.tile([64, 512], MDT, tag="projn", name=f"pn{n}")
                nc.vector.tensor_mul(projn[:], pacc_sb[0:64, :], bc_sb[:])
                fps = pj_ps.tile([64, 512], F32, tag="pj", name=f"fps{n}")
                nc.tensor.matmul(
                    fps[:], bo_sb[:], ones_sb[0:1, :], start=True, stop=False
                )
                nc.tensor.matmul(
                    fps[:], wo_sb[:], projn[:], start=False, stop=True
                )
                nc.vector.tensor_add(out_sb[:, qsl], fps[:], x_sb[0:64, qsl])
                nc.sync.dma_start(out_d.ap()[:, qsl], out_sb[:, qsl])

            # ---- software-pipelined attention: scores/exp of tile n
            # interleaved with attn@V of tile n-1; the previous tile's
            # normalize/out-proj ops are emitted a few batches in so the PE
            # never stalls on the softmax-denominator chain ----
            pending = None
            for n in range(4):
                for bi, batch in enumerate(batches):
                    emit_scores_batch(n, batch)
                    if n == 0 and bi < 8:
                        emit_v_pair(2 * bi)
                        emit_v_pair(2 * bi + 1)
                    if n > 0:
                        emit_proj_batch(n - 1, batch)
                    if bi == 2 and pending is not None:
                        emit_finish_b(*pending)
                        pending = None
                if n > 0:
                    pending = (n - 1,) + emit_finish_a(n - 1)
            for batch in batches:
                emit_proj_batch(3, batch)
                if pending is not None and batch[0] == batches[2][0]:
                    emit_finish_b(*pending)
                    pending = None
            emit_finish_b(3, *emit_finish_a(3))

    nc.compile()
    return nc


def host_prep(x, gamma, beta, Wq, bq, Wk, bk, Wv, bv, Wo, bo):
    """Build the 8 per-core input dicts."""
    f32 = lambda a: np.ascontiguousarray(np.asarray(a, np.float32))
    x = f32(x)
    gamma, beta = f32(gamma), f32(beta)
    Wq, Wk, Wv, Wo = f32(Wq), f32(Wk), f32(Wv), f32(Wo)
    bq, bk, bv, bo = f32(bq), f32(bk), f32(bv), f32(bo)

    wq_dup = np.ascontiguousarray(np.concatenate([Wq, Wq], axis=1))
    z = np.zeros((64, 64), np.float32)
    wk_blk = np.ascontiguousarray(np.block([[Wk, z], [z, Wk]]))
    wv_blk = np.ascontiguousarray(np.block([[Wv, z], [z, Wv]]))
    comb = np.zeros((128, 128), np.float32)
    comb[:64, :64] = 1.0 / 64.0
    comb[64:, 64:] = 1.0 / 64.0
    mdt_np = mybir.dt.np(MDT)
    m = lambda a: np.ascontiguousarray(a).astype(mdt_np)
    shared = {
        "wq": m(wq_dup), "wk": m(wk_blk), "wv": m(wv_blk), "wo": m(Wo),
        "bq": m(np.tile(bq, 2)[None]),
        "bk": m(np.tile(bk, 2)[None]),
        "bv": m(np.tile(bv, 2)[None]),
        "bo": m(bo[None]),
        "gam": np.ascontiguousarray(np.tile(gamma, 2)[:, None]),
        "bet": np.ascontiguousarray(np.tile(beta, 2)[:, None]),
        "comb": comb,
    }
    in_maps = []
    for core in range(8):
        b, h = core // 2, core % 2
        xT = x[b].reshape(HW, C).T  # [64, 4096]
        halves = xT.reshape(C, 2, HALF)[:, [h, 1 - h], :]
        xp = np.ascontiguousarray(halves.transpose(1, 0, 2).reshape(128, HALF))
        in_maps.append({"xp": xp, **shared})
    return in_maps


def assemble(results, dtype):
    out = np.empty((B, HW, C), np.float32)
    for core in range(8):
        b, h = core // 2, core % 2
        out[b, HALF * h : HALF * h + HALF] = results[core]["out"].T
    return out.reshape(B, H, W, C).astype(dtype, copy=False)


_NC_CACHE = []


def kernel(x, gamma, beta, Wq, bq, Wk, bk, Wv, bv, Wo, bo):
    from concourse.bass_utils import run_bass_kernel_spmd

    if not _NC_CACHE:
        _NC_CACHE.append(build_nc())
    nc = _NC_CACHE[0]
    in_maps = host_prep(x, gamma, beta, Wq, bq, Wk, bk, Wv, bv, Wo, bo)
    res = run_bass_kernel_spmd(nc, in_maps, core_ids=list(range(8)))
    return assemble(res.results, np.asarray(x).dtype)


if __name__ == "__main__":
    rng = np.random.default_rng(0)
    inputs = {
        "x": rng.standard_normal((B, H, W, C)).astype(np.float32),
        "gamma": np.ones(C, np.float32), "beta": np.zeros(C, np.float32),
        "Wq": (rng.standard_normal((C, C)) / 8).astype(np.float32),
        "bq": np.zeros(C, np.float32),
        "Wk": (rng.standard_normal((C, C)) / 8).astype(np.float32),
        "bk": np.zeros(C, np.float32),
        "Wv": (rng.standard_normal((C, C)) / 8).astype(np.float32),
        "bv": np.zeros(C, np.float32),
        "Wo": (rng.standard_normal((C, C)) / 8).astype(np.float32),
        "bo": np.zeros(C, np.float32),
    }
    out = kernel(**inputs)
    print("kernel ran, out shape", out.shape, out.dtype)
